# revision 11
# baseline (speedup 1.0000x reference)
"""GATv2 (2-layer, 4-head then 1-head, + linear head) on 8 Trainium2 NeuronCores.

Strategy (edge-parallel, dst-sharded):
  - Nodes are assigned to the 8 cores (snake-dealt by degree so every core sees a
    near-identical degree profile), then sorted per-core by (kL, kH) where
    kL/kH = number of in-edges whose source lives in cores 0-4 / 5-7. Cores 0-4
    occupy table rows [0, 5*PC) < 32768, so int16 gather indices work via a
    two-table split.
  - Each core computes the full node transform xl1' = x @ (W1l*att) (att folded
    into the weights, columns sign-grouped per head), gathers xl1'[src] for its
    edges with dma_gather, computes scores with Prelu ops (leaky-relu identity:
    att*lrelu(z) = prelu(u,0.2) for att>0, prelu(0.2u,5) for att<0, u=att*z),
    does the segment softmax fully on-chip (slots of one dst live in one
    partition row), and aggregates with a strided reduce. Layer-2 node features
    are exchanged with a single AllGather (each core transforms only its own
    h-shard).
"""
import sys
if "/opt/trn_rl_repo" not in sys.path:
    sys.path.insert(0, "/opt/trn_rl_repo")

import numpy as np

NCORES = 8
NLOW_CORES = 5  # cores 0..4 are the "low" gather table

F32 = None  # set lazily (mybir import is heavy; keep kernel importable anywhere)


# --------------------------------------------------------------------------
# Host-side graph preprocessing
# --------------------------------------------------------------------------

def _prep_graph(src_pos, dst_pos, N, PC, T):
    """Given edges in permuted-position space, build per-core gather/mask arrays.

    Returns tiles meta (kLp/kHp per tile, shared by all cores) and per-core
    idxL/idxH/mask arrays.
    """
    SPLIT = NLOW_CORES * PC
    E = len(src_pos)
    core_of_dst = dst_pos // PC
    row_of_dst = dst_pos % PC          # 0..PC-1 within the core
    is_high = (src_pos >= SPLIT)

    # group edges by (core, dst row, is_high), slot index within group
    key = (core_of_dst.astype(np.int64) * PC + row_of_dst) * 2 + is_high
    order = np.argsort(key, kind="stable")
    ks = key[order]
    # cumcount within group
    grp_start = np.r_[0, np.flatnonzero(np.diff(ks)) + 1]
    sizes = np.diff(np.r_[grp_start, E])
    slot = np.arange(E) - np.repeat(grp_start, sizes)

    e_core = core_of_dst[order]
    e_row = row_of_dst[order]
    e_high = is_high[order]
    e_src = src_pos[order]

    # per (core,row) kL / kH
    kL = np.zeros((NCORES, PC), np.int32)
    kH = np.zeros((NCORES, PC), np.int32)
    np.add.at(kL, (e_core[~e_high], e_row[~e_high]), 1)
    np.add.at(kH, (e_core[e_high], e_row[e_high]), 1)

    # tile maxes, shared across cores
    kLt = kL.reshape(NCORES, T, 128)
    kHt = kH.reshape(NCORES, T, 128)
    kLp = kLt.max(axis=(0, 2)).astype(np.int64)   # [T]
    kHp = kHt.max(axis=(0, 2)).astype(np.int64)
    Kt = kLp + kHp
    assert Kt.min() >= 1

    offL = np.r_[0, np.cumsum(kLp)]
    offH = np.r_[0, np.cumsum(kHp)]
    offM = np.r_[0, np.cumsum(Kt)]

    idxL = np.zeros((NCORES, int(offL[-1]), 128), np.int16)  # [core, slot-major, row]
    idxH = np.zeros((NCORES, int(offH[-1]), 128), np.int16)
    mask = np.zeros((NCORES, 128, int(offM[-1])), np.float32)

    tile_of_row = e_row // 128
    r128 = e_row % 128
    lo = ~e_high
    idxL[e_core[lo], offL[tile_of_row[lo]] + slot[lo], r128[lo]] = e_src[lo].astype(np.int16)
    idxH[e_core[~lo], offH[tile_of_row[~lo]] + slot[~lo], r128[~lo]] = (
        (e_src[~lo] - SPLIT).astype(np.int16))

    # mask: valid slots
    for t in range(T):
        mrows = np.arange(128)
        for c in range(NCORES):
            kLrow = kLt[c, t]
            kHrow = kHt[c, t]
            sl = np.arange(Kt[t])[None, :]
            m = (sl < kLrow[:, None]) | (
                (sl >= kLp[t]) & (sl < kLp[t] + kHrow[:, None]))
            mask[c, mrows, offM[t]:offM[t + 1]] = m.astype(np.float32)

    # wrap idx arrays: flat position p = slot*128 + row -> [16, num/16] rep to 128
    def wrap(a):  # a: [core, slots_total, 128]
        out = []
        for c in range(NCORES):
            fl = a[c].reshape(-1)  # slot-major within each tile? NO: global concat
            out.append(fl)
        return out

    # per-tile wrap (positions are per-gather-call)
    idxL_w = np.zeros((NCORES, 128, int(offL[-1]) * 8), np.int16)
    idxH_w = np.zeros((NCORES, 128, int(offH[-1]) * 8), np.int16)
    for c in range(NCORES):
        for t in range(T):
            for (src_arr, off_arr, dst_arr) in (
                    (idxL, offL, idxL_w), (idxH, offH, idxH_w)):
                kp = off_arr[t + 1] - off_arr[t]
                if kp == 0:
                    continue
                fl = src_arr[c, off_arr[t]:off_arr[t + 1], :].reshape(-1)  # [kp*128]
                w = fl.reshape(-1, 16).T  # [16, kp*8]
                dst_arr[c, :, off_arr[t] * 8:off_arr[t + 1] * 8] = np.tile(w, (8, 1))

    meta = dict(kLp=kLp, kHp=kHp, Kt=Kt, offL=offL, offH=offH, offM=offM,
                SPLIT=SPLIT)
    return meta, idxL_w, idxH_w, mask


def _sign_perm(att_flat, heads, hid):
    """Per-head permutation putting att>0 columns first. Returns perm, pos-counts."""
    perm = np.zeros(heads * hid, np.int64)
    pcnt = np.zeros(heads, np.int64)
    for h in range(heads):
        a = att_flat[h * hid:(h + 1) * hid]
        pos = np.flatnonzero(a > 0)
        neg = np.flatnonzero(a <= 0)
        perm[h * hid:(h + 1) * hid] = h * hid + np.r_[pos, neg]
        pcnt[h] = len(pos)
    return perm, pcnt


# --------------------------------------------------------------------------
# Device program
# --------------------------------------------------------------------------

def build_program(cfg):
    import os
    PH = int(os.environ.get("KPH", "9"))
    SUB = int(os.environ.get("KSUB", "99"))
    import concourse.mybir as mybir
    import concourse.bacc as bacc
    import concourse.tile as tile
    from concourse.masks import make_identity

    dt = mybir.dt
    AF = mybir.ActivationFunctionType
    ALU = mybir.AluOpType
    AX = mybir.AxisListType

    NG, PC, T = cfg["NG"], cfg["PC"], cfg["T"]
    SPLIT = cfg["SPLIT"]
    IN_DIM, HID, HEADS = cfg["IN_DIM"], cfg["HID"], cfg["HEADS"]
    F1 = HEADS * HID           # 256
    F2 = HID                   # 64
    OUT = cfg["OUT_DIM"]
    kLp, kHp, Kt = cfg["kLp"], cfg["kHp"], cfg["Kt"]
    offL, offH, offM = cfg["offL"], cfg["offH"], cfg["offM"]
    p1, p2 = cfg["p1"], cfg["p2"]          # per-head positive counts
    NT_G = NG // 128                        # global transform tiles
    GCH = 8       # gather chunk slots (<=1024 SWDGE ring descriptors)
    GCH2 = 8

    nc = bacc.Bacc("TRN2", target_bir_lowering=False, debug=False,
                   num_devices=NCORES)

    # ---- I/O ----
    xT = nc.dram_tensor("xT", [IN_DIM, NG], dt.float32, kind="ExternalInput")
    xTo = nc.dram_tensor("xTo", [IN_DIM, PC], dt.float32, kind="ExternalInput")
    w1l = nc.dram_tensor("w1l", [IN_DIM, F1], dt.float32, kind="ExternalInput")
    w1r = nc.dram_tensor("w1r", [IN_DIM, F1], dt.float32, kind="ExternalInput")
    w2l = nc.dram_tensor("w2l", [128, (F1 // 128) * F2], dt.float32, kind="ExternalInput")
    w2r = nc.dram_tensor("w2r", [128, (F1 // 128) * F2], dt.float32, kind="ExternalInput")
    consts = nc.dram_tensor("consts", [128, cfg["CW"]], dt.float32, kind="ExternalInput")
    idxL = nc.dram_tensor("idxL", [128, int(offL[-1]) * 8], dt.int16, kind="ExternalInput")
    idxH = nc.dram_tensor("idxH", [128, int(offH[-1]) * 8], dt.int16, kind="ExternalInput")
    maskd = nc.dram_tensor("maskd", [128, int(offM[-1])], dt.float32, kind="ExternalInput")
    out_d = nc.dram_tensor("out", [PC, OUT], dt.float32, kind="ExternalOutput")
    KDBG = int(os.environ.get("KDBG", "-1"))
    if KDBG >= 0:
        ktd = int(Kt[KDBG])
        dbg_g = nc.dram_tensor("dbg_g", [128, ktd, F1], dt.float32, kind="ExternalOutput")
        dbg_u = nc.dram_tensor("dbg_u", [128, ktd, F1], dt.float32, kind="ExternalOutput")
        dbg_sc = nc.dram_tensor("dbg_sc", [128, ktd, HEADS], dt.float32, kind="ExternalOutput")
        dbg_dn = nc.dram_tensor("dbg_dn", [128, HEADS], dt.float32, kind="ExternalOutput")
        dbg_ag = nc.dram_tensor("dbg_ag", [128, F1], dt.float32, kind="ExternalOutput")
        dbg_h = nc.dram_tensor("dbg_h", [128, F1], dt.float32, kind="ExternalOutput")

    # ---- internal DRAM ----
    xl1_d = nc.dram_tensor("xl1", [NG, F1], dt.bfloat16)
    hT_d = nc.dram_tensor("hT", [F1 // 128, 128, PC], dt.float32)
    cc_in = nc.dram_tensor("cc_in", [PC, F2], dt.float32)
    cc_out = nc.dram_tensor("cc_out", [NG, F2], dt.float32, addr_space="Shared")

    # const slices (columns in consts)
    oIA1, oB1 = 0, F1
    oIA2, oB2 = 2 * F1, 2 * F1 + F2
    oWL = 2 * F1 + 2 * F2
    oBL = oWL + OUT * F2

    S1 = F1 // 128  # h-feature slabs (2)

    with tile.TileContext(nc) as tc:
        with tc.tile_pool(name="const", bufs=1) as cpool, \
             tc.tile_pool(name="work", bufs=3) as wpool, \
             tc.tile_pool(name="edge", bufs=2) as epool, \
             tc.tile_pool(name="small", bufs=3) as spool, \
             tc.tile_pool(name="scrp", bufs=2) as scrpool, \
             tc.tile_pool(name="ps", bufs=2, space="PSUM") as pspool, \
             tc.tile_pool(name="ps2", bufs=2, space="PSUM") as ps2pool:

            # ---- load constants ----
            w1l_s = cpool.tile([128, F1], dt.float32, tag="w1l")
            w1r_s = cpool.tile([128, F1], dt.float32, tag="w1r")
            w2l_s = cpool.tile([128, S1 * F2], dt.float32, tag="w2l")
            w2r_s = cpool.tile([128, S1 * F2], dt.float32, tag="w2r")
            cst = cpool.tile([128, cfg["CW"]], dt.float32, tag="cst")
            ident = cpool.tile([128, 128], dt.float32, tag="ident")
            nc.sync.dma_start(out=w1l_s[:], in_=w1l.ap())
            nc.sync.dma_start(out=w1r_s[:], in_=w1r.ap())
            nc.sync.dma_start(out=w2l_s[:], in_=w2l.ap())
            nc.sync.dma_start(out=w2r_s[:], in_=w2r.ap())
            nc.sync.dma_start(out=cst[:], in_=consts.ap())
            make_identity(nc, ident[:])

            # ---- phase 1: xl1' for all NG nodes ----
            for t in range(NT_G):
                sl = slice(t * 128, (t + 1) * 128)
                xt = wpool.tile([128, 128], dt.float32, tag="xt")
                nc.sync.dma_start(out=xt[:], in_=xT.ap()[:, sl])
                ps = pspool.tile([128, F1], dt.float32, space="PSUM", tag="tr")
                nc.tensor.matmul(ps[:], lhsT=xt[:], rhs=w1l_s[:], start=True, stop=True)
                sb = wpool.tile([128, F1], dt.bfloat16, tag="xl1sb")
                nc.vector.tensor_copy(out=sb[:], in_=ps[:])
                nc.sync.dma_start(out=xl1_d.ap()[sl, :], in_=sb[:])

            tc.strict_bb_all_engine_barrier()

            # ---- phase 2+3: per-tile L1 edge processing ----
            lowT = xl1_d.ap()[0:SPLIT, :]
            highT = xl1_d.ap()[SPLIT:NG, :]
            for t in range(T if PH >= 2 else 0):
                kl, kh, kt = int(kLp[t]), int(kHp[t]), int(Kt[t])
                sl = slice(t * 128, (t + 1) * 128)
                # xr tile from own x
                xt = wpool.tile([128, 128], dt.float32, tag="xt")
                nc.sync.dma_start(out=xt[:], in_=xTo.ap()[:, sl])
                ps = pspool.tile([128, F1], dt.float32, space="PSUM", tag="tr")
                nc.tensor.matmul(ps[:], lhsT=xt[:], rhs=w1r_s[:], start=True, stop=True)
                xr = wpool.tile([128, F1], dt.float32, tag="xr")
                nc.vector.tensor_copy(out=xr[:], in_=ps[:])

                # gather xl1'[src]
                et = epool.tile([128, kt, F1], dt.bfloat16, tag="edge1")
                if SUB < 2: continue
                if kl > 0:
                    il = spool.tile([128, kl * 8], dt.int16, tag="il")
                    nc.sync.dma_start(out=il[:], in_=idxL.ap()[:, int(offL[t]) * 8:int(offL[t + 1]) * 8])
                    for c0 in range(0, kl, GCH):
                        w = min(GCH, kl - c0)
                        nc.gpsimd.dma_gather(et[:, c0:c0 + w, :], lowT,
                                             il[:, c0 * 8:(c0 + w) * 8],
                                             w * 128, w * 128, F1)
                if kh > 0:
                    ih = spool.tile([128, kh * 8], dt.int16, tag="ih")
                    nc.sync.dma_start(out=ih[:], in_=idxH.ap()[:, int(offH[t]) * 8:int(offH[t + 1]) * 8])
                    for c0 in range(0, kh, GCH):
                        w = min(GCH, kh - c0)
                        nc.gpsimd.dma_gather(et[:, kl + c0:kl + c0 + w, :], highT,
                                             ih[:, c0 * 8:(c0 + w) * 8],
                                             w * 128, w * 128, F1)
                mk = spool.tile([128, kt], dt.float32, tag="mk")
                nc.sync.dma_start(out=mk[:], in_=maskd.ap()[:, int(offM[t]):int(offM[t + 1])])

                if SUB < 3: continue
                # u = xl'[src] + xr'[dst]
                nc.vector.tensor_tensor(
                    out=et[:], in0=et[:],
                    in1=xr[:].rearrange("p (o f) -> p o f", o=1).to_broadcast([128, kt, F1]),
                    op=ALU.add)

                if SUB < 4: continue
                if t == KDBG:
                    nc.sync.dma_start(out=dbg_u.ap(), in_=et[:])
                # scores via Prelu + per-head reduce
                et4 = et[:].rearrange("p k (h d) -> p k h d", h=HEADS)
                sc = spool.tile([128, kt, HEADS], dt.float32, tag="sc")
                for h in range(HEADS):
                    ph = int(p1[h])
                    scr = scrpool.tile([128, kt, HID], dt.bfloat16, tag="scr")
                    if ph > 0:
                        nc.scalar.activation(scr[:, :, 0:ph], et4[:, :, h, 0:ph],
                                             AF.Prelu, alpha=0.2)
                    if ph < HID:
                        nc.scalar.activation(scr[:, :, ph:HID], et4[:, :, h, ph:HID],
                                             AF.Prelu, scale=0.2, alpha=5.0)
                    nc.vector.tensor_reduce(sc[:, :, h], scr[:],
                                            axis=AX.X, op=ALU.add)

                if SUB < 5: continue
                # w = exp(score) * mask ; denom; 1/denom
                nc.scalar.activation(sc[:], sc[:], AF.Exp)
                nc.vector.tensor_tensor(
                    out=sc[:], in0=sc[:],
                    in1=mk[:].rearrange("p (k o) -> p k o", o=1).to_broadcast([128, kt, HEADS]),
                    op=ALU.mult)
                if t == KDBG:
                    nc.sync.dma_start(out=dbg_sc.ap(), in_=sc[:])
                dn = spool.tile([128, HEADS], dt.float32, tag="dn")
                nc.vector.tensor_reduce(dn[:], sc[:].rearrange("p k h -> p h k"),
                                        axis=AX.X, op=ALU.add)
                nc.vector.tensor_scalar_add(dn[:], dn[:], 1e-12)
                rc = spool.tile([128, HEADS], dt.float32, tag="rc")
                nc.vector.reciprocal(rc[:], dn[:])
                if t == KDBG:
                    nc.sync.dma_start(out=dbg_dn.ap(), in_=dn[:])

                if SUB < 6: continue
                # v = xl'[src] * w ; aggregate over slots
                nc.vector.tensor_tensor(
                    out=et[:], in0=et[:],
                    in1=sc[:].rearrange("p k (h o) -> p k h o", o=1).to_broadcast([128, kt, HEADS, HID]),
                    op=ALU.mult)
                ag = wpool.tile([128, F1], dt.float32, tag="ag")
                nc.vector.tensor_reduce(
                    ag[:], et[:].rearrange("p k (h d) -> p h d k", h=HEADS),
                    axis=AX.X, op=ALU.add)

                if SUB < 7: continue
                # unscale: * (1/denom) per head, * inv_att, + b1, then ELU
                nc.vector.tensor_tensor(
                    out=ag[:], in0=ag[:].rearrange("p (h d) -> p h d", h=HEADS),
                    in1=rc[:].rearrange("p (h o) -> p h o", o=1).to_broadcast([128, HEADS, HID]),
                    op=ALU.mult)
                if t == KDBG:
                    nc.sync.dma_start(out=dbg_ag.ap(), in_=ag[:])
                nc.vector.tensor_sub(ag[:], ag[:], xr[:])
                nc.vector.tensor_mul(ag[:], ag[:], cst[:, oIA1:oIA1 + F1])
                nc.vector.tensor_add(ag[:], ag[:], cst[:, oB1:oB1 + F1])
                # elu: max(x,0)-1 + exp(min(x,0))
                t1 = wpool.tile([128, F1], dt.float32, tag="elu1")
                nc.vector.tensor_scalar(t1[:], ag[:], 0.0, None, ALU.min)
                nc.scalar.activation(t1[:], t1[:], AF.Exp)
                nc.vector.tensor_scalar(ag[:], ag[:], 0.0, -1.0, ALU.max, ALU.add)
                nc.vector.tensor_add(ag[:], ag[:], t1[:])

                if SUB < 8: continue
                if t == KDBG:
                    nc.sync.dma_start(out=dbg_h.ap(), in_=ag[:])
                # transpose h -> hT
                for s in range(S1):
                    tp = ps2pool.tile([128, 128], dt.float32, space="PSUM", tag="tp")
                    nc.tensor.transpose(out=tp[:], in_=ag[:, s * 128:(s + 1) * 128],
                                        identity=ident[:])
                    tb = wpool.tile([128, 128], dt.float32, tag="tb")
                    nc.vector.tensor_copy(out=tb[:], in_=tp[:])
                    nc.sync.dma_start(out=hT_d.ap()[s, :, sl], in_=tb[:])

            tc.strict_bb_all_engine_barrier()

            # ---- phase 4: L2 transforms from hT shard ----
            xr2 = cpool.tile([128, T, F2], dt.float32, tag="xr2all")
            for t in range(T if PH >= 3 else 0):
                sl = slice(t * 128, (t + 1) * 128)
                ps_l = ps2pool.tile([128, F2], dt.float32, space="PSUM", tag="l2l")
                ps_r = ps2pool.tile([128, F2], dt.float32, space="PSUM", tag="l2r")
                for s in range(S1):
                    ht = wpool.tile([128, 128], dt.float32, tag="ht")
                    nc.sync.dma_start(out=ht[:], in_=hT_d.ap()[s, :, sl])
                    nc.tensor.matmul(ps_l[:], lhsT=ht[:], rhs=w2l_s[:, s * F2:(s + 1) * F2],
                                     start=(s == 0), stop=(s == S1 - 1))
                    nc.tensor.matmul(ps_r[:], lhsT=ht[:], rhs=w2r_s[:, s * F2:(s + 1) * F2],
                                     start=(s == 0), stop=(s == S1 - 1))
                xl2 = wpool.tile([128, F2], dt.float32, tag="xl2")
                nc.vector.tensor_copy(out=xl2[:], in_=ps_l[:])
                nc.vector.tensor_copy(out=xr2[:, t, :], in_=ps_r[:])
                nc.sync.dma_start(out=cc_in.ap()[sl, :], in_=xl2[:])

            tc.strict_bb_all_engine_barrier()
            if PH >= 4:
              nc.gpsimd.collective_compute(
                "AllGather", mybir.AluOpType.bypass,
                replica_groups=[list(range(NCORES))],
                ins=[cc_in.ap().opt()], outs=[cc_out.ap().opt()])
            tc.strict_bb_all_engine_barrier()

            # ---- phase 5: L2 edge processing + final linear ----
            lowT2 = cc_out.ap()[0:SPLIT, :]
            highT2 = cc_out.ap()[SPLIT:NG, :]
            pp2 = int(p2[0])
            for t in range(T if PH >= 5 else 0):
                kl, kh, kt = int(kLp[t]), int(kHp[t]), int(Kt[t])
                sl = slice(t * 128, (t + 1) * 128)
                et = epool.tile([128, kt, F2], dt.float32, tag="edge2")
                if kl > 0:
                    il = spool.tile([128, kl * 8], dt.int16, tag="il")
                    nc.sync.dma_start(out=il[:], in_=idxL.ap()[:, int(offL[t]) * 8:int(offL[t + 1]) * 8])
                    for c0 in range(0, kl, GCH2):
                        w = min(GCH2, kl - c0)
                        nc.gpsimd.dma_gather(et[:, c0:c0 + w, :], lowT2,
                                             il[:, c0 * 8:(c0 + w) * 8],
                                             w * 128, w * 128, F2)
                if kh > 0:
                    ih = spool.tile([128, kh * 8], dt.int16, tag="ih")
                    nc.sync.dma_start(out=ih[:], in_=idxH.ap()[:, int(offH[t]) * 8:int(offH[t + 1]) * 8])
                    for c0 in range(0, kh, GCH2):
                        w = min(GCH2, kh - c0)
                        nc.gpsimd.dma_gather(et[:, kl + c0:kl + c0 + w, :], highT2,
                                             ih[:, c0 * 8:(c0 + w) * 8],
                                             w * 128, w * 128, F2)
                mk = spool.tile([128, kt], dt.float32, tag="mk")
                nc.sync.dma_start(out=mk[:], in_=maskd.ap()[:, int(offM[t]):int(offM[t + 1])])

                nc.vector.tensor_tensor(
                    out=et[:], in0=et[:],
                    in1=xr2[:, t, :].rearrange("p (o f) -> p o f", o=1).to_broadcast([128, kt, F2]),
                    op=ALU.add)

                scr = scrpool.tile([128, kt, F2], dt.float32, tag="scr2")
                if pp2 > 0:
                    nc.scalar.activation(scr[:, :, 0:pp2], et[:, :, 0:pp2],
                                         AF.Prelu, alpha=0.2)
                if pp2 < F2:
                    nc.scalar.activation(scr[:, :, pp2:F2], et[:, :, pp2:F2],
                                         AF.Prelu, scale=0.2, alpha=5.0)
                sc = spool.tile([128, kt], dt.float32, tag="sc2")
                nc.vector.tensor_reduce(sc[:], scr[:], axis=AX.X, op=ALU.add)
                nc.scalar.activation(sc[:], sc[:], AF.Exp)
                nc.vector.tensor_mul(sc[:], sc[:], mk[:])
                dn = spool.tile([128, 1], dt.float32, tag="dn2")
                nc.vector.tensor_reduce(dn[:], sc[:], axis=AX.X, op=ALU.add)
                nc.vector.tensor_scalar_add(dn[:], dn[:], 1e-12)
                rc = spool.tile([128, 1], dt.float32, tag="rc2")
                nc.vector.reciprocal(rc[:], dn[:])

                nc.vector.tensor_tensor(
                    out=et[:], in0=et[:],
                    in1=sc[:].rearrange("p (k o) -> p k o", o=1).to_broadcast([128, kt, F2]),
                    op=ALU.mult)
                ag = wpool.tile([128, F2], dt.float32, tag="ag2")
                nc.vector.tensor_reduce(
                    ag[:], et[:].rearrange("p k f -> p f k"), axis=AX.X, op=ALU.add)

                nc.vector.tensor_scalar_mul(ag[:], ag[:], rc[:, 0:1])
                nc.vector.tensor_sub(ag[:], ag[:], xr2[:, t, :])
                nc.vector.tensor_mul(ag[:], ag[:], cst[:, oIA2:oIA2 + F2])
                nc.vector.tensor_add(ag[:], ag[:], cst[:, oB2:oB2 + F2])
                t1 = wpool.tile([128, F2], dt.float32, tag="elu2")
                nc.vector.tensor_scalar(t1[:], ag[:], 0.0, None, ALU.min)
                nc.scalar.activation(t1[:], t1[:], AF.Exp)
                nc.vector.tensor_scalar(ag[:], ag[:], 0.0, -1.0, ALU.max, ALU.add)
                nc.vector.tensor_add(ag[:], ag[:], t1[:])

                # final linear
                ot = spool.tile([128, OUT], dt.float32, tag="ot")
                tmp = wpool.tile([128, F2], dt.float32, tag="fl")
                for c in range(OUT):
                    nc.vector.tensor_mul(tmp[:], ag[:], cst[:, oWL + c * F2:oWL + (c + 1) * F2])
                    nc.vector.tensor_reduce(ot[:, c:c + 1], tmp[:], axis=AX.X, op=ALU.add)
                nc.vector.tensor_add(ot[:], ot[:], cst[:, oBL:oBL + OUT])
                nc.sync.dma_start(out=out_d.ap()[sl, :], in_=ot[:])

    nc.compile()
    return nc


# --------------------------------------------------------------------------
# kernel() entry point
# --------------------------------------------------------------------------

_CACHE = {}


def kernel(x, edge_index, W1l, W1r, att1, b1, W2l, W2r, att2, b2, Wlin, blin):
    from concourse import bass_utils

    x = np.asarray(x, np.float32)
    N, IN_DIM = x.shape
    HEADS, HID = np.asarray(att1).shape
    F1 = HEADS * HID
    OUT_DIM = np.asarray(Wlin).shape[1]
    E = edge_index.shape[1]
    src = np.asarray(edge_index[0], np.int64)
    dst = np.asarray(edge_index[1], np.int64)

    PC = -(-(-(-N // -NCORES)) // -128) * 128  # ceil(ceil(N/8)/128)*128
    PC = ((N + NCORES - 1) // NCORES + 127) // 128 * 128
    T = PC // 128
    NG = NCORES * PC
    SPLIT = NLOW_CORES * PC
    assert SPLIT < 32768 and NG - SPLIT < 32768

    # ---- node -> core assignment + permutation ----
    deg = np.bincount(dst, minlength=N)
    assert deg.min() >= 1, "degree-0 nodes present; kernel assumes none"
    order0 = np.argsort(deg, kind="stable")
    core_of = np.empty(N, np.int64)
    core_of[order0] = np.arange(N) % NCORES
    # low/high source
    is_low_src = core_of[src] < NLOW_CORES
    kL0 = np.zeros(N, np.int64)
    kH0 = np.zeros(N, np.int64)
    np.add.at(kL0, dst[is_low_src], 1)
    np.add.at(kH0, dst[~is_low_src], 1)
    # per-core sort by (kL, kH)
    perm_lists = []
    pos_of = np.empty(N, np.int64)
    for c in range(NCORES):
        nodes = np.flatnonzero(core_of == c)
        o = np.lexsort((kL0[nodes], kL0[nodes] + kH0[nodes]))
        nodes = nodes[o]
        perm_lists.append(nodes)
        pos_of[nodes] = c * PC + np.arange(len(nodes))

    src_pos = pos_of[src]
    dst_pos = pos_of[dst]

    meta, idxL_w, idxH_w, mask = _prep_graph(src_pos, dst_pos, N, PC, T)

    # ---- weights (host-side param prep: att folding + sign-grouping) ----
    att1_f = np.asarray(att1, np.float64).reshape(-1)
    att2_f = np.asarray(att2, np.float64).reshape(-1)
    assert np.abs(att1_f).min() > 1e-12 and np.abs(att2_f).min() > 1e-12
    pi1, p1 = _sign_perm(att1_f, HEADS, HID)
    pi2, p2 = _sign_perm(att2_f, 1, HID)

    W1l_f = (np.asarray(W1l, np.float64) * att1_f[None, :])[:, pi1].astype(np.float32)
    W1r_f = (np.asarray(W1r, np.float64) * att1_f[None, :])[:, pi1].astype(np.float32)
    inv1 = (1.0 / att1_f)[pi1].astype(np.float32)
    b1_p = np.asarray(b1, np.float32)[pi1]
    W2l_f = ((np.asarray(W2l, np.float64)[pi1, :]) * att2_f[None, :])[:, pi2].astype(np.float32)
    W2r_f = ((np.asarray(W2r, np.float64)[pi1, :]) * att2_f[None, :])[:, pi2].astype(np.float32)
    inv2 = (1.0 / att2_f)[pi2].astype(np.float32)
    b2_p = np.asarray(b2, np.float32)[pi2]
    Wlin_p = np.asarray(Wlin, np.float32)[pi2, :]
    blin_p = np.asarray(blin, np.float32)

    S1 = F1 // 128
    w2l_dev = W2l_f.reshape(S1, 128, HID).transpose(1, 0, 2).reshape(128, S1 * HID)
    w2r_dev = W2r_f.reshape(S1, 128, HID).transpose(1, 0, 2).reshape(128, S1 * HID)

    CW = 2 * F1 + 2 * HID + OUT_DIM * HID + OUT_DIM
    consts = np.zeros((128, CW), np.float32)
    consts[:, 0:F1] = inv1[None, :]
    consts[:, F1:2 * F1] = b1_p[None, :]
    consts[:, 2 * F1:2 * F1 + HID] = inv2[None, :]
    consts[:, 2 * F1 + HID:2 * F1 + 2 * HID] = b2_p[None, :]
    for c in range(OUT_DIM):
        consts[:, 2 * F1 + 2 * HID + c * HID:2 * F1 + 2 * HID + (c + 1) * HID] = Wlin_p[:, c][None, :]
    consts[:, 2 * F1 + 2 * HID + OUT_DIM * HID:] = blin_p[None, :]

    # permuted x, padded + transposed
    x_perm = np.zeros((NG, IN_DIM), np.float32)
    for c in range(NCORES):
        nodes = perm_lists[c]
        x_perm[c * PC:c * PC + len(nodes)] = x[nodes]
    xT_full = np.ascontiguousarray(x_perm.T)

    cfg = dict(NG=NG, PC=PC, T=T, SPLIT=SPLIT, IN_DIM=IN_DIM, HID=HID,
               HEADS=HEADS, OUT_DIM=OUT_DIM, CW=CW,
               kLp=meta["kLp"], kHp=meta["kHp"], Kt=meta["Kt"],
               offL=meta["offL"], offH=meta["offH"], offM=meta["offM"],
               p1=p1, p2=p2)

    key = (N, E, IN_DIM, HID, HEADS, OUT_DIM,
           tuple(cfg["kLp"]), tuple(cfg["kHp"]), tuple(p1), tuple(p2))
    if key not in _CACHE:
        _CACHE[key] = build_program(cfg)
    nc = _CACHE[key]

    in_maps = []
    for c in range(NCORES):
        in_maps.append({
            "xT": xT_full,
            "xTo": np.ascontiguousarray(xT_full[:, c * PC:(c + 1) * PC]),
            "w1l": W1l_f, "w1r": W1r_f, "w2l": w2l_dev, "w2r": w2r_dev,
            "consts": consts,
            "idxL": idxL_w[c], "idxH": idxH_w[c], "maskd": mask[c],
        })

    res = bass_utils.run_bass_kernel_spmd(nc, in_maps, core_ids=list(range(NCORES)))
    kernel._last = dict(res=res.results, in_maps=in_maps, nc=nc,
                        perm_lists=perm_lists, pos_of=pos_of,
                        cfg=cfg, meta=meta, W1l_f=W1l_f, W1r_f=W1r_f,
                        pi1=pi1, pi2=pi2, inv1=inv1, x_perm=x_perm,
                        idxL_w=idxL_w, idxH_w=idxH_w, mask=mask)

    out = np.empty((N, OUT_DIM), np.float32)
    for c in range(NCORES):
        nodes = perm_lists[c]
        out[nodes] = res.results[c]["out"][:len(nodes)]
    return out


# revision 23
# speedup vs baseline: 1.0637x; 1.0637x over previous
"""GATv2 (2-layer, 4-head then 1-head, + linear head) on 8 Trainium2 NeuronCores.

Strategy (edge-parallel, dst-sharded):
  - Nodes are assigned to the 8 cores (snake-dealt by degree so every core sees a
    near-identical degree profile), then sorted per-core by (kL, kH) where
    kL/kH = number of in-edges whose source lives in cores 0-4 / 5-7. Cores 0-4
    occupy table rows [0, 5*PC) < 32768, so int16 gather indices work via a
    two-table split.
  - Each core computes the full node transform xl1' = x @ (W1l*att) (att folded
    into the weights, columns sign-grouped per head), gathers xl1'[src] for its
    edges with dma_gather, computes scores with Prelu ops (leaky-relu identity:
    att*lrelu(z) = prelu(u,0.2) for att>0, prelu(0.2u,5) for att<0, u=att*z),
    does the segment softmax fully on-chip (slots of one dst live in one
    partition row), and aggregates with a strided reduce. Layer-2 node features
    are exchanged with a single AllGather (each core transforms only its own
    h-shard).
"""
import sys
if "/opt/trn_rl_repo" not in sys.path:
    sys.path.insert(0, "/opt/trn_rl_repo")

import numpy as np

NCORES = 8
NLOW_CORES = 5  # cores 0..4 are the "low" gather table

F32 = None  # set lazily (mybir import is heavy; keep kernel importable anywhere)


# --------------------------------------------------------------------------
# Host-side graph preprocessing
# --------------------------------------------------------------------------

def _prep_graph(src_pos, dst_pos, N, PC, T):
    """Given edges in permuted-position space, build per-core gather/mask arrays.

    Returns tiles meta (kLp/kHp per tile, shared by all cores) and per-core
    idxL/idxH/mask arrays.
    """
    SPLIT = NLOW_CORES * PC
    E = len(src_pos)
    core_of_dst = dst_pos // PC
    row_of_dst = dst_pos % PC          # 0..PC-1 within the core
    is_high = (src_pos >= SPLIT)

    # group edges by (core, dst row, is_high), slot index within group
    key = (core_of_dst.astype(np.int64) * PC + row_of_dst) * 2 + is_high
    order = np.argsort(key, kind="stable")
    ks = key[order]
    # cumcount within group
    grp_start = np.r_[0, np.flatnonzero(np.diff(ks)) + 1]
    sizes = np.diff(np.r_[grp_start, E])
    slot = np.arange(E) - np.repeat(grp_start, sizes)

    e_core = core_of_dst[order]
    e_row = row_of_dst[order]
    e_high = is_high[order]
    e_src = src_pos[order]

    # per (core,row) kL / kH
    kL = np.zeros((NCORES, PC), np.int32)
    kH = np.zeros((NCORES, PC), np.int32)
    np.add.at(kL, (e_core[~e_high], e_row[~e_high]), 1)
    np.add.at(kH, (e_core[e_high], e_row[e_high]), 1)

    # tile maxes, shared across cores
    kLt = kL.reshape(NCORES, T, 128)
    kHt = kH.reshape(NCORES, T, 128)
    kLp = kLt.max(axis=(0, 2)).astype(np.int64)   # [T]
    kHp = kHt.max(axis=(0, 2)).astype(np.int64)
    Kt = kLp + kHp
    assert Kt.min() >= 1

    offL = np.r_[0, np.cumsum(kLp)]
    offH = np.r_[0, np.cumsum(kHp)]
    offM = np.r_[0, np.cumsum(Kt)]

    idxL = np.zeros((NCORES, int(offL[-1]), 128), np.int16)  # [core, slot-major, row]
    idxH = np.zeros((NCORES, int(offH[-1]), 128), np.int16)
    mask = np.zeros((NCORES, 128, int(offM[-1])), np.float32)

    tile_of_row = e_row // 128
    r128 = e_row % 128
    lo = ~e_high
    idxL[e_core[lo], offL[tile_of_row[lo]] + slot[lo], r128[lo]] = e_src[lo].astype(np.int16)
    idxH[e_core[~lo], offH[tile_of_row[~lo]] + slot[~lo], r128[~lo]] = (
        (e_src[~lo] - SPLIT).astype(np.int16))

    # mask: valid slots
    for t in range(T):
        mrows = np.arange(128)
        for c in range(NCORES):
            kLrow = kLt[c, t]
            kHrow = kHt[c, t]
            sl = np.arange(Kt[t])[None, :]
            m = (sl < kLrow[:, None]) | (
                (sl >= kLp[t]) & (sl < kLp[t] + kHrow[:, None]))
            mask[c, mrows, offM[t]:offM[t + 1]] = m.astype(np.float32)

    # wrap idx arrays: flat position p = slot*128 + row -> [16, num/16] rep to 128
    def wrap(a):  # a: [core, slots_total, 128]
        out = []
        for c in range(NCORES):
            fl = a[c].reshape(-1)  # slot-major within each tile? NO: global concat
            out.append(fl)
        return out

    # pack per-tile aux: [idxL wrap | idxH wrap | mask bf16-bitcast] int16
    import ml_dtypes
    Wt = 8 * kLp + 8 * kHp + Kt
    offA = np.r_[0, np.cumsum(Wt)]
    aux = np.zeros((NCORES, 128, int(offA[-1])), np.int16)
    for c in range(NCORES):
        for t in range(T):
            a0 = int(offA[t])
            for (src_arr, off_arr) in ((idxL, offL), (idxH, offH)):
                kp = int(off_arr[t + 1] - off_arr[t])
                if kp:
                    fl = src_arr[c, off_arr[t]:off_arr[t + 1], :].reshape(-1)
                    w = fl.reshape(-1, 16).T  # [16, kp*8]
                    aux[c, :, a0:a0 + kp * 8] = np.tile(w, (8, 1))
                a0 += kp * 8
            mbf = mask[c, :, offM[t]:offM[t + 1]].astype(ml_dtypes.bfloat16)
            aux[c, :, a0:a0 + int(Kt[t])] = mbf.view(np.int16)

    meta = dict(kLp=kLp, kHp=kHp, Kt=Kt, offL=offL, offH=offH, offM=offM,
                offA=offA, SPLIT=SPLIT)
    return meta, aux


def _sign_perm(att_flat, heads, hid):
    """Per-head permutation putting att>0 columns first. Returns perm, pos-counts."""
    perm = np.zeros(heads * hid, np.int64)
    pcnt = np.zeros(heads, np.int64)
    for h in range(heads):
        a = att_flat[h * hid:(h + 1) * hid]
        pos = np.flatnonzero(a > 0)
        neg = np.flatnonzero(a <= 0)
        perm[h * hid:(h + 1) * hid] = h * hid + np.r_[pos, neg]
        pcnt[h] = len(pos)
    return perm, pcnt


# --------------------------------------------------------------------------
# Device program
# --------------------------------------------------------------------------

def build_program(cfg):
    import os
    PH = int(os.environ.get("KPH", "9"))
    SUB = int(os.environ.get("KSUB", "99"))
    import concourse.mybir as mybir
    import concourse.bacc as bacc
    import concourse.tile as tile
    from concourse.masks import make_identity

    dt = mybir.dt
    AF = mybir.ActivationFunctionType
    ALU = mybir.AluOpType
    AX = mybir.AxisListType

    NG, PC, T = cfg["NG"], cfg["PC"], cfg["T"]
    SPLIT = cfg["SPLIT"]
    IN_DIM, HID, HEADS = cfg["IN_DIM"], cfg["HID"], cfg["HEADS"]
    F1 = HEADS * HID           # 256
    F2 = HID                   # 64
    OUT = cfg["OUT_DIM"]
    kLp, kHp, Kt = cfg["kLp"], cfg["kHp"], cfg["Kt"]
    offL, offH, offM = cfg["offL"], cfg["offH"], cfg["offM"]
    p1, p2 = cfg["p1"], cfg["p2"]          # per-head positive counts
    NT_G = NG // 128                        # global transform tiles
    GCH = 8       # gather chunk slots (<=1024 SWDGE ring descriptors)
    GCH2 = 8

    nc = bacc.Bacc("TRN2", target_bir_lowering=False, debug=False,
                   num_devices=NCORES)

    # ---- I/O ----
    xT = nc.dram_tensor("xT", [IN_DIM, NG], dt.float32, kind="ExternalInput")
    xTo = nc.dram_tensor("xTo", [IN_DIM, PC], dt.float32, kind="ExternalInput")
    w1l = nc.dram_tensor("w1l", [IN_DIM, F1], dt.float32, kind="ExternalInput")
    w1r = nc.dram_tensor("w1r", [IN_DIM, F1], dt.float32, kind="ExternalInput")
    w2l = nc.dram_tensor("w2l", [128, (F1 // 128) * F2], dt.float32, kind="ExternalInput")
    w2r = nc.dram_tensor("w2r", [128, (F1 // 128) * F2], dt.float32, kind="ExternalInput")
    consts = nc.dram_tensor("consts", [128, cfg["CW"]], dt.float32, kind="ExternalInput")
    offA = cfg["offA"]
    auxd = nc.dram_tensor("auxd", [128, int(offA[-1])], dt.int16, kind="ExternalInput")
    out_d = nc.dram_tensor("out", [PC, OUT], dt.float32, kind="ExternalOutput")
    KDBG = int(os.environ.get("KDBG", "-1"))
    if KDBG >= 0:
        ktd = int(Kt[KDBG])
        dbg_g = nc.dram_tensor("dbg_g", [128, ktd, F1], dt.float32, kind="ExternalOutput")
        dbg_u = nc.dram_tensor("dbg_u", [128, ktd, F1], dt.float32, kind="ExternalOutput")
        dbg_sc = nc.dram_tensor("dbg_sc", [128, ktd, HEADS], dt.float32, kind="ExternalOutput")
        dbg_dn = nc.dram_tensor("dbg_dn", [128, HEADS], dt.float32, kind="ExternalOutput")
        dbg_ag = nc.dram_tensor("dbg_ag", [128, F1], dt.float32, kind="ExternalOutput")
        dbg_h = nc.dram_tensor("dbg_h", [128, F1], dt.float32, kind="ExternalOutput")

    # ---- internal DRAM ----
    xl1_d = nc.dram_tensor("xl1", [NG, F1], dt.bfloat16)
    cc_in = nc.dram_tensor("cc_in", [PC, F2], dt.float32)
    cc_out = nc.dram_tensor("cc_out", [NG, F2], dt.float32, addr_space="Shared")
    # chunked all-gather scratch (one per chunk)
    NCHUNK = min(4, T)
    cb = [0]
    for g in range(NCHUNK):
        cb.append(min(T, (T * (g + 1) + NCHUNK - 1) // NCHUNK))
    cc_sc = [nc.dram_tensor(f"cc_sc{g}", [NCORES * (cb[g + 1] - cb[g]) * 128, F2],
                            dt.float32, addr_space="Shared")
             for g in range(NCHUNK)]

    # const slices (columns in consts)
    oIA1, oB1 = 0, F1
    oIA2, oB2 = 2 * F1, 2 * F1 + F2
    oWL = 2 * F1 + 2 * F2
    oBL = oWL + OUT * F2

    S1 = F1 // 128  # h-feature slabs (2)

    with tile.TileContext(nc) as tc:
        with tc.tile_pool(name="const", bufs=1) as cpool, \
             tc.tile_pool(name="work", bufs=3) as wpool, \
             tc.tile_pool(name="edge", bufs=2) as epool, \
             tc.tile_pool(name="edge2p", bufs=3) as e2pool, \
             tc.tile_pool(name="small", bufs=3) as spool, \
             tc.tile_pool(name="scrp", bufs=2) as scrpool, \
             tc.tile_pool(name="ps", bufs=2, space="PSUM") as pspool, \
             tc.tile_pool(name="ps2", bufs=2, space="PSUM") as ps2pool:

            # ---- load constants ----
            w1l_s = cpool.tile([128, F1], dt.float32r, tag="w1l")
            w1r_s = cpool.tile([128, F1], dt.float32r, tag="w1r")
            w2l_s = cpool.tile([128, S1 * F2], dt.float32, tag="w2l")
            w2r_s = cpool.tile([128, S1 * F2], dt.float32, tag="w2r")
            cst = cpool.tile([128, cfg["CW"]], dt.float32, tag="cst")
            ident = cpool.tile([128, 128], dt.float32, tag="ident")
            nc.sync.dma_start(out=w1l_s[:], in_=w1l.ap().bitcast(dt.float32r))
            nc.sync.dma_start(out=w1r_s[:], in_=w1r.ap().bitcast(dt.float32r))
            nc.sync.dma_start(out=w2l_s[:], in_=w2l.ap())
            nc.sync.dma_start(out=w2r_s[:], in_=w2r.ap())
            nc.sync.dma_start(out=cst[:], in_=consts.ap())
            make_identity(nc, ident[:])

            _regs = {}
            def nreg(v):
                if v not in _regs:
                    _regs[v] = nc.gpsimd.to_reg(v)
                return _regs[v]

            # ---- phase 1: xl1' for all NG nodes (4 node-tiles per chunk) ----
            CH = 4
            assert NT_G % CH == 0
            for t in range(NT_G // CH):
                sl = slice(t * 128 * CH, (t + 1) * 128 * CH)
                xt = wpool.tile([128, CH * 128], dt.float32r, tag="xt")
                nc.sync.dma_start(out=xt[:], in_=xT.ap()[:, sl].bitcast(dt.float32r))
                sb = wpool.tile([128, CH, F1], dt.bfloat16, tag="xl1sb")
                for i in range(CH):
                    ps = pspool.tile([128, F1], dt.float32, space="PSUM", tag="tr")
                    nc.tensor.matmul(ps[:], lhsT=xt[:, i * 128:(i + 1) * 128],
                                     rhs=w1l_s[:], start=True, stop=True)
                    if i % 2 == 0:
                        nc.vector.tensor_copy(out=sb[:, i, :], in_=ps[:])
                    else:
                        nc.scalar.copy(out=sb[:, i, :], in_=ps[:])
                nc.scalar.dma_start(
                    out=xl1_d.ap()[sl, :].rearrange("(s p) f -> p s f", p=128),
                    in_=sb[:])

            tc.strict_bb_all_engine_barrier()

            # ---- phase 2+3: per-tile L1 edge processing (+ fused L2 transform) ----
            xr2 = cpool.tile([128, T, F2], dt.float32, tag="xr2all")
            lowT = xl1_d.ap()[0:SPLIT, :]
            highT = xl1_d.ap()[SPLIT:NG, :]
            for t in range(T if PH >= 2 else 0):
                kl, kh, kt = int(kLp[t]), int(kHp[t]), int(Kt[t])
                sl = slice(t * 128, (t + 1) * 128)
                # xr tile from own x
                xt = wpool.tile([128, 128], dt.float32r, tag="xt")
                nc.sync.dma_start(out=xt[:], in_=xTo.ap()[:, sl].bitcast(dt.float32r))
                ps = pspool.tile([128, F1], dt.float32, space="PSUM", tag="tr")
                nc.tensor.matmul(ps[:], lhsT=xt[:], rhs=w1r_s[:], start=True, stop=True)
                xr = wpool.tile([128, F1], dt.bfloat16, tag="xr")
                nc.vector.tensor_copy(out=xr[:], in_=ps[:])

                # gather xl1'[src]
                et = epool.tile([128, kt, F1], dt.bfloat16, tag="edge1")
                ax = spool.tile([128, 8 * kl + 8 * kh + kt], dt.int16, tag="ax")
                nc.sync.dma_start(out=ax[:], in_=auxd.ap()[:, int(offA[t]):int(offA[t + 1])])
                il = ax[:, 0:8 * kl]
                ih = ax[:, 8 * kl:8 * (kl + kh)]
                mk = ax[:, 8 * (kl + kh):].bitcast(dt.bfloat16)
                for c0 in range(0, kl, GCH):
                    w = min(GCH, kl - c0)
                    nc.gpsimd.dma_gather(et[:, c0:c0 + w, :], lowT,
                                         il[:, c0 * 8:(c0 + w) * 8],
                                         w * 128, nreg(w * 128), F1)
                for c0 in range(0, kh, GCH):
                    w = min(GCH, kh - c0)
                    nc.gpsimd.dma_gather(et[:, kl + c0:kl + c0 + w, :], highT,
                                         ih[:, c0 * 8:(c0 + w) * 8],
                                         w * 128, nreg(w * 128), F1)

                # u = xl'[src] + xr'[dst]
                nc.vector.tensor_tensor(
                    out=et[:], in0=et[:],
                    in1=xr[:].rearrange("p (o f) -> p o f", o=1).to_broadcast([128, kt, F1]),
                    op=ALU.add)

                # scores via Prelu + per-head reduce
                et4 = et[:].rearrange("p k (h d) -> p k h d", h=HEADS)
                sc = spool.tile([128, kt, HEADS], dt.bfloat16, tag="sc")
                for h in range(HEADS):
                    ph = int(p1[h])
                    scr = scrpool.tile([128, kt, HID], dt.bfloat16, tag="scr")
                    if ph > 0:
                        nc.scalar.activation(scr[:, :, 0:ph], et4[:, :, h, 0:ph],
                                             AF.Prelu, alpha=0.2)
                    if ph < HID:
                        nc.scalar.activation(scr[:, :, ph:HID], et4[:, :, h, ph:HID],
                                             AF.Prelu, scale=0.2, alpha=5.0)
                    with nc.allow_low_precision(reason="bf16 scores"):
                        d = HID
                        while d > 16:
                            d //= 2
                            nc.vector.tensor_tensor(
                                out=scr[:, :, 0:d], in0=scr[:, :, 0:d],
                                in1=scr[:, :, d:2 * d], op=ALU.add)
                        nc.vector.tensor_reduce(sc[:, :, h], scr[:, :, 0:d],
                                                axis=AX.X, op=ALU.add)

                # w = exp(score) * mask ; denom; 1/denom
                nc.scalar.activation(sc[:], sc[:], AF.Exp)
                nc.vector.tensor_tensor(
                    out=sc[:], in0=sc[:],
                    in1=mk[:].rearrange("p (k o) -> p k o", o=1).to_broadcast([128, kt, HEADS]),
                    op=ALU.mult)
                dn = spool.tile([128, HEADS], dt.float32, tag="dn")
                nc.vector.tensor_reduce(dn[:], sc[:].rearrange("p k h -> p h k"),
                                        axis=AX.X, op=ALU.add)
                nc.vector.tensor_scalar_add(dn[:], dn[:], 1e-12)
                rc = spool.tile([128, HEADS], dt.float32, tag="rc")
                nc.vector.reciprocal(rc[:], dn[:])

                # v = xl'[src] * w ; aggregate over slots
                # duplicate w into adjacent pairs so the bcast AP's innermost
                # dim is packed (step 1 x2) -> DVE 2x mode stays enabled
                scp = spool.tile([128, kt, HEADS, 2], dt.bfloat16, tag="scp")
                nc.vector.tensor_copy(
                    out=scp[:],
                    in_=sc[:].rearrange("p k (h o) -> p k h o", o=1).to_broadcast([128, kt, HEADS, 2]))
                nc.vector.tensor_tensor(
                    out=et[:].rearrange("p k (h d e) -> p k h d e", h=HEADS, e=2),
                    in0=et[:].rearrange("p k (h d e) -> p k h d e", h=HEADS, e=2),
                    in1=scp[:].rearrange("p k h (o e) -> p k h o e", o=1).to_broadcast([128, kt, HEADS, HID // 2, 2]),
                    op=ALU.mult)
                ag = wpool.tile([128, F1], dt.float32, tag="ag")
                cur = kt
                with nc.allow_low_precision(reason="bf16 agg tree"):
                    while cur > 3:
                        hh = (cur + 1) // 2
                        nc.vector.tensor_tensor(
                            out=et[:, 0:cur - hh, :], in0=et[:, 0:cur - hh, :],
                            in1=et[:, hh:cur, :], op=ALU.add)
                        cur = hh
                nc.vector.tensor_reduce(
                    ag[:], et[:, 0:cur, :].rearrange("p k (h d) -> p h d k", h=HEADS),
                    axis=AX.X, op=ALU.add)

                # unscale: * (1/denom) per head, * inv_att, + b1, then ELU
                nc.vector.tensor_tensor(
                    out=ag[:], in0=ag[:].rearrange("p (h d) -> p h d", h=HEADS),
                    in1=rc[:].rearrange("p (h o) -> p h o", o=1).to_broadcast([128, HEADS, HID]),
                    op=ALU.mult)
                nc.gpsimd.tensor_sub(ag[:], ag[:], xr[:])
                nc.gpsimd.tensor_mul(ag[:], ag[:], cst[:, oIA1:oIA1 + F1])
                nc.gpsimd.tensor_add(ag[:], ag[:], cst[:, oB1:oB1 + F1])
                # elu: max(x,0)-1 + exp(min(x,0))
                t1 = wpool.tile([128, F1], dt.float32, tag="elu1")
                nc.vector.tensor_scalar(t1[:], ag[:], 0.0, None, ALU.min)
                nc.scalar.activation(t1[:], t1[:], AF.Exp)
                nc.vector.tensor_scalar(ag[:], ag[:], 0.0, -1.0, ALU.max, ALU.add)
                nc.vector.tensor_add(ag[:], ag[:], t1[:])

                # transpose h -> hT tiles (SBUF) and L2 transforms right here
                tb = wpool.tile([128, S1, 128], dt.float32, tag="tb")
                for s in range(S1):
                    tp = ps2pool.tile([128, 128], dt.float32, space="PSUM", tag="tp")
                    nc.tensor.transpose(out=tp[:], in_=ag[:, s * 128:(s + 1) * 128],
                                        identity=ident[:])
                    nc.scalar.copy(out=tb[:, s, :], in_=tp[:])
                ps_l = ps2pool.tile([128, F2], dt.float32, space="PSUM", tag="l2l")
                ps_r = ps2pool.tile([128, F2], dt.float32, space="PSUM", tag="l2r")
                for s in range(S1):
                    nc.tensor.matmul(ps_l[:], lhsT=tb[:, s, :], rhs=w2l_s[:, s * F2:(s + 1) * F2],
                                     start=(s == 0), stop=(s == S1 - 1))
                    nc.tensor.matmul(ps_r[:], lhsT=tb[:, s, :], rhs=w2r_s[:, s * F2:(s + 1) * F2],
                                     start=(s == 0), stop=(s == S1 - 1))
                xl2 = wpool.tile([128, F2], dt.float32, tag="xl2")
                nc.scalar.copy(out=xl2[:], in_=ps_l[:])
                nc.vector.tensor_copy(out=xr2[:, t, :], in_=ps_r[:])
                nc.scalar.dma_start(out=cc_in.ap()[sl, :], in_=xl2[:])

                # chunk boundary: kick off the partial AllGather
                for g in range(NCHUNK):
                    if t == cb[g + 1] - 1:
                        r0, r1 = cb[g] * 128, cb[g + 1] * 128
                        if os.environ.get("KNOCC") == "1":
                            cbt = wpool.tile([128, F2], dt.float32, tag="ccstub")
                            for tt in range(cb[g], cb[g + 1]):
                                ssl = slice(tt * 128, (tt + 1) * 128)
                                nc.sync.dma_start(out=cbt[:], in_=cc_in.ap()[ssl, :])
                                nc.sync.dma_start(
                                    out=cc_sc[g].ap()[(tt - cb[g]) * 128:(tt - cb[g] + 1) * 128, :],
                                    in_=cbt[:])
                        else:
                            nc.gpsimd.collective_compute(
                                "AllGather", mybir.AluOpType.bypass,
                                replica_groups=[list(range(NCORES))],
                                ins=[cc_in.ap()[r0:r1, :].opt()],
                                outs=[cc_sc[g].ap().opt()])
                        rows = (cb[g + 1] - cb[g]) * 128
                        for c in range(NCORES):
                            nc.sync.dma_start(
                                out=cc_out.ap()[c * PC + r0:c * PC + r0 + rows, :],
                                in_=cc_sc[g].ap()[c * rows:(c + 1) * rows, :])

            tc.strict_bb_all_engine_barrier()

            # ---- phase 5: L2 edge processing + final linear ----
            lowT2 = cc_out.ap()[0:SPLIT, :]
            highT2 = cc_out.ap()[SPLIT:NG, :]
            pp2 = int(p2[0])
            for t in range(T if PH >= 5 else 0):
                kl, kh, kt = int(kLp[t]), int(kHp[t]), int(Kt[t])
                sl = slice(t * 128, (t + 1) * 128)
                et = e2pool.tile([128, kt, F2], dt.float32, tag="edge2")
                ax = spool.tile([128, 8 * kl + 8 * kh + kt], dt.int16, tag="ax")
                nc.sync.dma_start(out=ax[:], in_=auxd.ap()[:, int(offA[t]):int(offA[t + 1])])
                il = ax[:, 0:8 * kl]
                ih = ax[:, 8 * kl:8 * (kl + kh)]
                mk = ax[:, 8 * (kl + kh):].bitcast(dt.bfloat16)
                for c0 in range(0, kl, GCH2):
                    w = min(GCH2, kl - c0)
                    nc.gpsimd.dma_gather(et[:, c0:c0 + w, :], lowT2,
                                         il[:, c0 * 8:(c0 + w) * 8],
                                         w * 128, nreg(w * 128), F2)
                for c0 in range(0, kh, GCH2):
                    w = min(GCH2, kh - c0)
                    nc.gpsimd.dma_gather(et[:, kl + c0:kl + c0 + w, :], highT2,
                                         ih[:, c0 * 8:(c0 + w) * 8],
                                         w * 128, nreg(w * 128), F2)

                etb = e2pool.tile([128, kt, F2], dt.bfloat16, tag="edge2b")
                nc.vector.tensor_copy(out=etb[:], in_=et[:])
                xr2b = spool.tile([128, F2], dt.bfloat16, tag="xr2b")
                nc.vector.tensor_copy(out=xr2b[:], in_=xr2[:, t, :])
                nc.vector.tensor_tensor(
                    out=etb[:], in0=etb[:],
                    in1=xr2b[:].rearrange("p (o f) -> p o f", o=1).to_broadcast([128, kt, F2]),
                    op=ALU.add)

                scr = scrpool.tile([128, kt, F2], dt.bfloat16, tag="scr2")
                if pp2 > 0:
                    nc.scalar.activation(scr[:, :, 0:pp2], etb[:, :, 0:pp2],
                                         AF.Prelu, alpha=0.2)
                if pp2 < F2:
                    nc.scalar.activation(scr[:, :, pp2:F2], etb[:, :, pp2:F2],
                                         AF.Prelu, scale=0.2, alpha=5.0)
                sc = spool.tile([128, kt], dt.float32, tag="sc2")
                with nc.allow_low_precision(reason="bf16 l2 scores"):
                    d = F2
                    while d > 16:
                        d //= 2
                        nc.vector.tensor_tensor(
                            out=scr[:, :, 0:d], in0=scr[:, :, 0:d],
                            in1=scr[:, :, d:2 * d], op=ALU.add)
                nc.vector.tensor_reduce(sc[:], scr[:, :, 0:16], axis=AX.X, op=ALU.add)
                nc.scalar.activation(sc[:], sc[:], AF.Exp)
                nc.vector.tensor_mul(sc[:], sc[:], mk[:])
                dn = spool.tile([128, 1], dt.float32, tag="dn2")
                nc.vector.tensor_reduce(dn[:], sc[:], axis=AX.X, op=ALU.add)
                nc.vector.tensor_scalar_add(dn[:], dn[:], 1e-12)
                rc = spool.tile([128, 1], dt.float32, tag="rc2")
                nc.vector.reciprocal(rc[:], dn[:])

                scp = spool.tile([128, kt, 2], dt.bfloat16, tag="scp2")
                nc.vector.tensor_copy(
                    out=scp[:],
                    in_=sc[:].rearrange("p (k o) -> p k o", o=1).to_broadcast([128, kt, 2]))
                nc.vector.tensor_tensor(
                    out=etb[:].rearrange("p k (d e) -> p k d e", e=2),
                    in0=etb[:].rearrange("p k (d e) -> p k d e", e=2),
                    in1=scp[:].rearrange("p k (o e) -> p k o e", o=1).to_broadcast([128, kt, F2 // 2, 2]),
                    op=ALU.mult)
                ag = wpool.tile([128, F2], dt.float32, tag="ag2")
                cur = kt
                with nc.allow_low_precision(reason="bf16 l2 agg tree"):
                    while cur > 3:
                        hh = (cur + 1) // 2
                        nc.vector.tensor_tensor(
                            out=etb[:, 0:cur - hh, :], in0=etb[:, 0:cur - hh, :],
                            in1=etb[:, hh:cur, :], op=ALU.add)
                        cur = hh
                nc.vector.tensor_reduce(
                    ag[:], etb[:, 0:cur, :].rearrange("p k f -> p f k"), axis=AX.X, op=ALU.add)

                nc.vector.tensor_scalar_mul(ag[:], ag[:], rc[:, 0:1])
                nc.gpsimd.tensor_sub(ag[:], ag[:], xr2b[:])
                nc.gpsimd.tensor_mul(ag[:], ag[:], cst[:, oIA2:oIA2 + F2])
                nc.gpsimd.tensor_add(ag[:], ag[:], cst[:, oB2:oB2 + F2])
                t1 = wpool.tile([128, F2], dt.float32, tag="elu2")
                nc.vector.tensor_scalar(t1[:], ag[:], 0.0, None, ALU.min)
                nc.scalar.activation(t1[:], t1[:], AF.Exp)
                nc.vector.tensor_scalar(ag[:], ag[:], 0.0, -1.0, ALU.max, ALU.add)
                nc.vector.tensor_add(ag[:], ag[:], t1[:])

                # final linear
                ot = spool.tile([128, OUT], dt.float32, tag="ot")
                tmp = wpool.tile([128, F2], dt.float32, tag="fl")
                for c in range(OUT):
                    nc.vector.tensor_mul(tmp[:], ag[:], cst[:, oWL + c * F2:oWL + (c + 1) * F2])
                    nc.vector.tensor_reduce(ot[:, c:c + 1], tmp[:], axis=AX.X, op=ALU.add)
                nc.vector.tensor_add(ot[:], ot[:], cst[:, oBL:oBL + OUT])
                nc.scalar.dma_start(out=out_d.ap()[sl, :], in_=ot[:])

    nc.compile()
    return nc


# --------------------------------------------------------------------------
# kernel() entry point
# --------------------------------------------------------------------------

_CACHE = {}


def kernel(x, edge_index, W1l, W1r, att1, b1, W2l, W2r, att2, b2, Wlin, blin):
    from concourse import bass_utils

    x = np.asarray(x, np.float32)
    N, IN_DIM = x.shape
    HEADS, HID = np.asarray(att1).shape
    F1 = HEADS * HID
    OUT_DIM = np.asarray(Wlin).shape[1]
    E = edge_index.shape[1]
    src = np.asarray(edge_index[0], np.int64)
    dst = np.asarray(edge_index[1], np.int64)

    PC = -(-(-(-N // -NCORES)) // -128) * 128  # ceil(ceil(N/8)/128)*128
    PC = ((N + NCORES - 1) // NCORES + 127) // 128 * 128
    T = PC // 128
    NG = NCORES * PC
    SPLIT = NLOW_CORES * PC
    assert SPLIT < 32768 and NG - SPLIT < 32768

    # ---- node -> core assignment + permutation ----
    deg = np.bincount(dst, minlength=N)
    assert deg.min() >= 1, "degree-0 nodes present; kernel assumes none"
    order0 = np.argsort(deg, kind="stable")
    core_of = np.empty(N, np.int64)
    core_of[order0] = np.arange(N) % NCORES
    # low/high source
    is_low_src = core_of[src] < NLOW_CORES
    kL0 = np.zeros(N, np.int64)
    kH0 = np.zeros(N, np.int64)
    np.add.at(kL0, dst[is_low_src], 1)
    np.add.at(kH0, dst[~is_low_src], 1)
    # per-core sort by (kL, kH)
    perm_lists = []
    pos_of = np.empty(N, np.int64)
    for c in range(NCORES):
        nodes = np.flatnonzero(core_of == c)
        o = np.lexsort((kL0[nodes], kL0[nodes] + kH0[nodes]))
        nodes = nodes[o]
        perm_lists.append(nodes)
        pos_of[nodes] = c * PC + np.arange(len(nodes))

    src_pos = pos_of[src]
    dst_pos = pos_of[dst]

    meta, aux = _prep_graph(src_pos, dst_pos, N, PC, T)

    # ---- weights (host-side param prep: att folding + sign-grouping) ----
    att1_f = np.asarray(att1, np.float64).reshape(-1)
    att2_f = np.asarray(att2, np.float64).reshape(-1)
    assert np.abs(att1_f).min() > 1e-12 and np.abs(att2_f).min() > 1e-12
    pi1, p1 = _sign_perm(att1_f, HEADS, HID)
    pi2, p2 = _sign_perm(att2_f, 1, HID)

    W1l_f = (np.asarray(W1l, np.float64) * att1_f[None, :])[:, pi1].astype(np.float32)
    W1r_f = (np.asarray(W1r, np.float64) * att1_f[None, :])[:, pi1].astype(np.float32)
    inv1 = (1.0 / att1_f)[pi1].astype(np.float32)
    b1_p = np.asarray(b1, np.float32)[pi1]
    W2l_f = ((np.asarray(W2l, np.float64)[pi1, :]) * att2_f[None, :])[:, pi2].astype(np.float32)
    W2r_f = ((np.asarray(W2r, np.float64)[pi1, :]) * att2_f[None, :])[:, pi2].astype(np.float32)
    inv2 = (1.0 / att2_f)[pi2].astype(np.float32)
    b2_p = np.asarray(b2, np.float32)[pi2]
    Wlin_p = np.asarray(Wlin, np.float32)[pi2, :]
    blin_p = np.asarray(blin, np.float32)

    S1 = F1 // 128
    w2l_dev = W2l_f.reshape(S1, 128, HID).transpose(1, 0, 2).reshape(128, S1 * HID)
    w2r_dev = W2r_f.reshape(S1, 128, HID).transpose(1, 0, 2).reshape(128, S1 * HID)

    CW = 2 * F1 + 2 * HID + OUT_DIM * HID + OUT_DIM
    consts = np.zeros((128, CW), np.float32)
    consts[:, 0:F1] = inv1[None, :]
    consts[:, F1:2 * F1] = b1_p[None, :]
    consts[:, 2 * F1:2 * F1 + HID] = inv2[None, :]
    consts[:, 2 * F1 + HID:2 * F1 + 2 * HID] = b2_p[None, :]
    for c in range(OUT_DIM):
        consts[:, 2 * F1 + 2 * HID + c * HID:2 * F1 + 2 * HID + (c + 1) * HID] = Wlin_p[:, c][None, :]
    consts[:, 2 * F1 + 2 * HID + OUT_DIM * HID:] = blin_p[None, :]

    # permuted x, padded + transposed
    x_perm = np.zeros((NG, IN_DIM), np.float32)
    for c in range(NCORES):
        nodes = perm_lists[c]
        x_perm[c * PC:c * PC + len(nodes)] = x[nodes]
    xT_full = np.ascontiguousarray(x_perm.T)

    cfg = dict(NG=NG, PC=PC, T=T, SPLIT=SPLIT, IN_DIM=IN_DIM, HID=HID,
               HEADS=HEADS, OUT_DIM=OUT_DIM, CW=CW,
               kLp=meta["kLp"], kHp=meta["kHp"], Kt=meta["Kt"],
               offL=meta["offL"], offH=meta["offH"], offM=meta["offM"],
               offA=meta["offA"], p1=p1, p2=p2)

    key = (N, E, IN_DIM, HID, HEADS, OUT_DIM,
           tuple(cfg["kLp"]), tuple(cfg["kHp"]), tuple(p1), tuple(p2))
    if key not in _CACHE:
        _CACHE[key] = build_program(cfg)
    nc = _CACHE[key]

    in_maps = []
    for c in range(NCORES):
        in_maps.append({
            "xT": xT_full,
            "xTo": np.ascontiguousarray(xT_full[:, c * PC:(c + 1) * PC]),
            "w1l": W1l_f, "w1r": W1r_f, "w2l": w2l_dev, "w2r": w2r_dev,
            "consts": consts,
            "auxd": aux[c],
        })

    res = bass_utils.run_bass_kernel_spmd(nc, in_maps, core_ids=list(range(NCORES)))
    kernel._last = dict(res=res.results, in_maps=in_maps, nc=nc,
                        perm_lists=perm_lists, pos_of=pos_of,
                        cfg=cfg, meta=meta, W1l_f=W1l_f, W1r_f=W1r_f,
                        pi1=pi1, pi2=pi2, inv1=inv1, x_perm=x_perm, aux=aux)

    out = np.empty((N, OUT_DIM), np.float32)
    for c in range(NCORES):
        nodes = perm_lists[c]
        out[nodes] = res.results[c]["out"][:len(nodes)]
    return out


# revision 25
# speedup vs baseline: 1.0741x; 1.0098x over previous
"""GATv2 (2-layer, 4-head then 1-head, + linear head) on 8 Trainium2 NeuronCores.

Strategy (edge-parallel, dst-sharded):
  - Nodes are assigned to the 8 cores (snake-dealt by degree so every core sees a
    near-identical degree profile), then sorted per-core by (kL, kH) where
    kL/kH = number of in-edges whose source lives in cores 0-4 / 5-7. Cores 0-4
    occupy table rows [0, 5*PC) < 32768, so int16 gather indices work via a
    two-table split.
  - Each core computes the full node transform xl1' = x @ (W1l*att) (att folded
    into the weights, columns sign-grouped per head), gathers xl1'[src] for its
    edges with dma_gather, computes scores with Prelu ops (leaky-relu identity:
    att*lrelu(z) = prelu(u,0.2) for att>0, prelu(0.2u,5) for att<0, u=att*z),
    does the segment softmax fully on-chip (slots of one dst live in one
    partition row), and aggregates with a strided reduce. Layer-2 node features
    are exchanged with a single AllGather (each core transforms only its own
    h-shard).
"""
import sys
if "/opt/trn_rl_repo" not in sys.path:
    sys.path.insert(0, "/opt/trn_rl_repo")

import numpy as np

NCORES = 8
NLOW_CORES = 5  # cores 0..4 are the "low" gather table

F32 = None  # set lazily (mybir import is heavy; keep kernel importable anywhere)


# --------------------------------------------------------------------------
# Host-side graph preprocessing
# --------------------------------------------------------------------------

def _prep_graph(src_pos, dst_pos, N, PC, T):
    """Given edges in permuted-position space, build per-core gather/mask arrays.

    Returns tiles meta (kLp/kHp per tile, shared by all cores) and per-core
    idxL/idxH/mask arrays.
    """
    SPLIT = NLOW_CORES * PC
    E = len(src_pos)
    core_of_dst = dst_pos // PC
    row_of_dst = dst_pos % PC          # 0..PC-1 within the core
    is_high = (src_pos >= SPLIT)

    # group edges by (core, dst row, is_high), slot index within group
    key = (core_of_dst.astype(np.int64) * PC + row_of_dst) * 2 + is_high
    order = np.argsort(key, kind="stable")
    ks = key[order]
    # cumcount within group
    grp_start = np.r_[0, np.flatnonzero(np.diff(ks)) + 1]
    sizes = np.diff(np.r_[grp_start, E])
    slot = np.arange(E) - np.repeat(grp_start, sizes)

    e_core = core_of_dst[order]
    e_row = row_of_dst[order]
    e_high = is_high[order]
    e_src = src_pos[order]

    # per (core,row) kL / kH
    kL = np.zeros((NCORES, PC), np.int32)
    kH = np.zeros((NCORES, PC), np.int32)
    np.add.at(kL, (e_core[~e_high], e_row[~e_high]), 1)
    np.add.at(kH, (e_core[e_high], e_row[e_high]), 1)

    # tile maxes, shared across cores
    kLt = kL.reshape(NCORES, T, 128)
    kHt = kH.reshape(NCORES, T, 128)
    kLp = kLt.max(axis=(0, 2)).astype(np.int64)   # [T]
    kHp = kHt.max(axis=(0, 2)).astype(np.int64)
    Kt = kLp + kHp
    assert Kt.min() >= 1

    offL = np.r_[0, np.cumsum(kLp)]
    offH = np.r_[0, np.cumsum(kHp)]
    offM = np.r_[0, np.cumsum(Kt)]

    idxL = np.zeros((NCORES, int(offL[-1]), 128), np.int16)  # [core, slot-major, row]
    idxH = np.zeros((NCORES, int(offH[-1]), 128), np.int16)
    mask = np.zeros((NCORES, 128, int(offM[-1])), np.float32)

    tile_of_row = e_row // 128
    r128 = e_row % 128
    lo = ~e_high
    idxL[e_core[lo], offL[tile_of_row[lo]] + slot[lo], r128[lo]] = e_src[lo].astype(np.int16)
    idxH[e_core[~lo], offH[tile_of_row[~lo]] + slot[~lo], r128[~lo]] = (
        (e_src[~lo] - SPLIT).astype(np.int16))

    # mask: valid slots
    for t in range(T):
        mrows = np.arange(128)
        for c in range(NCORES):
            kLrow = kLt[c, t]
            kHrow = kHt[c, t]
            sl = np.arange(Kt[t])[None, :]
            m = (sl < kLrow[:, None]) | (
                (sl >= kLp[t]) & (sl < kLp[t] + kHrow[:, None]))
            mask[c, mrows, offM[t]:offM[t + 1]] = m.astype(np.float32)

    # wrap idx arrays: flat position p = slot*128 + row -> [16, num/16] rep to 128
    def wrap(a):  # a: [core, slots_total, 128]
        out = []
        for c in range(NCORES):
            fl = a[c].reshape(-1)  # slot-major within each tile? NO: global concat
            out.append(fl)
        return out

    # pack per-tile aux: [idxL wrap | idxH wrap | mask bf16-bitcast] int16
    import ml_dtypes
    Wt = 8 * kLp + 8 * kHp + Kt
    offA = np.r_[0, np.cumsum(Wt)]
    aux = np.zeros((NCORES, 128, int(offA[-1])), np.int16)
    for c in range(NCORES):
        for t in range(T):
            a0 = int(offA[t])
            for (src_arr, off_arr) in ((idxL, offL), (idxH, offH)):
                kp = int(off_arr[t + 1] - off_arr[t])
                if kp:
                    fl = src_arr[c, off_arr[t]:off_arr[t + 1], :].reshape(-1)
                    w = fl.reshape(-1, 16).T  # [16, kp*8]
                    aux[c, :, a0:a0 + kp * 8] = np.tile(w, (8, 1))
                a0 += kp * 8
            mbf = mask[c, :, offM[t]:offM[t + 1]].astype(ml_dtypes.bfloat16)
            aux[c, :, a0:a0 + int(Kt[t])] = mbf.view(np.int16)

    meta = dict(kLp=kLp, kHp=kHp, Kt=Kt, offL=offL, offH=offH, offM=offM,
                offA=offA, SPLIT=SPLIT)
    return meta, aux


def _sign_perm(att_flat, heads, hid):
    """Per-head permutation putting att>0 columns first. Returns perm, pos-counts."""
    perm = np.zeros(heads * hid, np.int64)
    pcnt = np.zeros(heads, np.int64)
    for h in range(heads):
        a = att_flat[h * hid:(h + 1) * hid]
        pos = np.flatnonzero(a > 0)
        neg = np.flatnonzero(a <= 0)
        perm[h * hid:(h + 1) * hid] = h * hid + np.r_[pos, neg]
        pcnt[h] = len(pos)
    return perm, pcnt


# --------------------------------------------------------------------------
# Device program
# --------------------------------------------------------------------------

def build_program(cfg):
    import os
    PH = int(os.environ.get("KPH", "9"))
    SUB = int(os.environ.get("KSUB", "99"))
    import concourse.mybir as mybir
    import concourse.bacc as bacc
    import concourse.tile as tile
    from concourse.masks import make_identity

    dt = mybir.dt
    AF = mybir.ActivationFunctionType
    ALU = mybir.AluOpType
    AX = mybir.AxisListType

    NG, PC, T = cfg["NG"], cfg["PC"], cfg["T"]
    SPLIT = cfg["SPLIT"]
    IN_DIM, HID, HEADS = cfg["IN_DIM"], cfg["HID"], cfg["HEADS"]
    F1 = HEADS * HID           # 256
    F2 = HID                   # 64
    OUT = cfg["OUT_DIM"]
    kLp, kHp, Kt = cfg["kLp"], cfg["kHp"], cfg["Kt"]
    offL, offH, offM = cfg["offL"], cfg["offH"], cfg["offM"]
    p1, p2 = cfg["p1"], cfg["p2"]          # per-head positive counts
    NT_G = NG // 128                        # global transform tiles
    GCH = 8       # gather chunk slots (<=1024 SWDGE ring descriptors)
    GCH2 = 8

    nc = bacc.Bacc("TRN2", target_bir_lowering=False, debug=False,
                   num_devices=NCORES)

    # ---- I/O ----
    xT = nc.dram_tensor("xT", [IN_DIM, NG], dt.float32, kind="ExternalInput")
    xTo = nc.dram_tensor("xTo", [IN_DIM, PC], dt.float32, kind="ExternalInput")
    w1l = nc.dram_tensor("w1l", [IN_DIM, F1], dt.float32, kind="ExternalInput")
    w1r = nc.dram_tensor("w1r", [IN_DIM, F1], dt.float32, kind="ExternalInput")
    w2l = nc.dram_tensor("w2l", [128, (F1 // 128) * F2], dt.float32, kind="ExternalInput")
    w2r = nc.dram_tensor("w2r", [128, (F1 // 128) * F2], dt.float32, kind="ExternalInput")
    consts = nc.dram_tensor("consts", [128, cfg["CW"]], dt.float32, kind="ExternalInput")
    offA = cfg["offA"]
    auxd = nc.dram_tensor("auxd", [128, int(offA[-1])], dt.int16, kind="ExternalInput")
    out_d = nc.dram_tensor("out", [PC, OUT], dt.float32, kind="ExternalOutput")
    KDBG = int(os.environ.get("KDBG", "-1"))
    if KDBG >= 0:
        ktd = int(Kt[KDBG])
        dbg_g = nc.dram_tensor("dbg_g", [128, ktd, F1], dt.float32, kind="ExternalOutput")
        dbg_u = nc.dram_tensor("dbg_u", [128, ktd, F1], dt.float32, kind="ExternalOutput")
        dbg_sc = nc.dram_tensor("dbg_sc", [128, ktd, HEADS], dt.float32, kind="ExternalOutput")
        dbg_dn = nc.dram_tensor("dbg_dn", [128, HEADS], dt.float32, kind="ExternalOutput")
        dbg_ag = nc.dram_tensor("dbg_ag", [128, F1], dt.float32, kind="ExternalOutput")
        dbg_h = nc.dram_tensor("dbg_h", [128, F1], dt.float32, kind="ExternalOutput")

    # ---- internal DRAM ----
    xl1_d = nc.dram_tensor("xl1", [NG, F1], dt.bfloat16)
    cc_in = nc.dram_tensor("cc_in", [PC, F2], dt.float32)
    cc_out = nc.dram_tensor("cc_out", [NG, F2], dt.float32, addr_space="Shared")
    # chunked all-gather scratch (one per chunk)
    NCHUNK = min(4, T)
    cb = [0]
    for g in range(NCHUNK):
        cb.append(min(T, (T * (g + 1) + NCHUNK - 1) // NCHUNK))
    cc_sc = [nc.dram_tensor(f"cc_sc{g}", [NCORES * (cb[g + 1] - cb[g]) * 128, F2],
                            dt.float32, addr_space="Shared")
             for g in range(NCHUNK)]

    # const slices (columns in consts)
    oIA1, oB1 = 0, F1
    oIA2, oB2 = 2 * F1, 2 * F1 + F2
    oWL = 2 * F1 + 2 * F2
    oBL = oWL + OUT * F2

    S1 = F1 // 128  # h-feature slabs (2)

    with tile.TileContext(nc) as tc:
        with tc.tile_pool(name="const", bufs=1) as cpool, \
             tc.tile_pool(name="work", bufs=3) as wpool, \
             tc.tile_pool(name="edge", bufs=2) as epool, \
             tc.tile_pool(name="edge2p", bufs=3) as e2pool, \
             tc.tile_pool(name="small", bufs=3) as spool, \
             tc.tile_pool(name="scrp", bufs=2) as scrpool, \
             tc.tile_pool(name="ps", bufs=2, space="PSUM") as pspool, \
             tc.tile_pool(name="ps2", bufs=2, space="PSUM") as ps2pool:

            # ---- load constants ----
            w1l_s = cpool.tile([128, F1], dt.float32r, tag="w1l")
            w1r_s = cpool.tile([128, F1], dt.float32r, tag="w1r")
            w2l_s = cpool.tile([128, S1 * F2], dt.float32, tag="w2l")
            w2r_s = cpool.tile([128, S1 * F2], dt.float32, tag="w2r")
            cst = cpool.tile([128, cfg["CW"]], dt.float32, tag="cst")
            ident = cpool.tile([128, 128], dt.float32, tag="ident")
            nc.sync.dma_start(out=w1l_s[:], in_=w1l.ap().bitcast(dt.float32r))
            nc.sync.dma_start(out=w1r_s[:], in_=w1r.ap().bitcast(dt.float32r))
            nc.sync.dma_start(out=w2l_s[:], in_=w2l.ap())
            nc.sync.dma_start(out=w2r_s[:], in_=w2r.ap())
            nc.sync.dma_start(out=cst[:], in_=consts.ap())
            make_identity(nc, ident[:])

            _regs = {}
            def nreg(v):
                if v not in _regs:
                    _regs[v] = nc.gpsimd.to_reg(v)
                return _regs[v]

            # ---- phase 1: xl1' for all NG nodes (4 node-tiles per chunk) ----
            CH = 4
            assert NT_G % CH == 0
            for t in range(NT_G // CH):
                sl = slice(t * 128 * CH, (t + 1) * 128 * CH)
                xt = wpool.tile([128, CH * 128], dt.float32r, tag="xt")
                nc.sync.dma_start(out=xt[:], in_=xT.ap()[:, sl].bitcast(dt.float32r))
                sb = wpool.tile([128, CH, F1], dt.bfloat16, tag="xl1sb")
                for i in range(CH):
                    ps = pspool.tile([128, F1], dt.float32, space="PSUM", tag="tr")
                    nc.tensor.matmul(ps[:], lhsT=xt[:, i * 128:(i + 1) * 128],
                                     rhs=w1l_s[:], start=True, stop=True)
                    if i % 2 == 0:
                        nc.vector.tensor_copy(out=sb[:, i, :], in_=ps[:])
                    else:
                        nc.scalar.copy(out=sb[:, i, :], in_=ps[:])
                nc.scalar.dma_start(
                    out=xl1_d.ap()[sl, :].rearrange("(s p) f -> p s f", p=128),
                    in_=sb[:])

            tc.strict_bb_all_engine_barrier()

            # ---- phase 2+3: per-tile L1 edge processing (+ fused L2 transform) ----
            xr2 = cpool.tile([128, T, F2], dt.float32, tag="xr2all")
            lowT = xl1_d.ap()[0:SPLIT, :]
            highT = xl1_d.ap()[SPLIT:NG, :]
            for t in range(T if PH >= 2 else 0):
                kl, kh, kt = int(kLp[t]), int(kHp[t]), int(Kt[t])
                sl = slice(t * 128, (t + 1) * 128)
                # xr tile from own x
                xt = wpool.tile([128, 128], dt.float32r, tag="xt")
                nc.sync.dma_start(out=xt[:], in_=xTo.ap()[:, sl].bitcast(dt.float32r))
                ps = pspool.tile([128, F1], dt.float32, space="PSUM", tag="tr")
                nc.tensor.matmul(ps[:], lhsT=xt[:], rhs=w1r_s[:], start=True, stop=True)
                xr = wpool.tile([128, F1], dt.bfloat16, tag="xr")
                nc.vector.tensor_copy(out=xr[:], in_=ps[:])

                # gather xl1'[src]
                et = epool.tile([128, kt, F1], dt.bfloat16, tag="edge1")
                ax = spool.tile([128, 8 * kl + 8 * kh + kt], dt.int16, tag="ax")
                nc.sync.dma_start(out=ax[:], in_=auxd.ap()[:, int(offA[t]):int(offA[t + 1])])
                il = ax[:, 0:8 * kl]
                ih = ax[:, 8 * kl:8 * (kl + kh)]
                mk = ax[:, 8 * (kl + kh):].bitcast(dt.bfloat16)
                for c0 in range(0, kl, GCH):
                    w = min(GCH, kl - c0)
                    nc.gpsimd.dma_gather(et[:, c0:c0 + w, :], lowT,
                                         il[:, c0 * 8:(c0 + w) * 8],
                                         w * 128, nreg(w * 128), F1)
                for c0 in range(0, kh, GCH):
                    w = min(GCH, kh - c0)
                    nc.gpsimd.dma_gather(et[:, kl + c0:kl + c0 + w, :], highT,
                                         ih[:, c0 * 8:(c0 + w) * 8],
                                         w * 128, nreg(w * 128), F1)

                # u = xl'[src] + xr'[dst]
                nc.vector.tensor_tensor(
                    out=et[:], in0=et[:],
                    in1=xr[:].rearrange("p (o f) -> p o f", o=1).to_broadcast([128, kt, F1]),
                    op=ALU.add)

                # scores via Prelu + per-head reduce
                et4 = et[:].rearrange("p k (h d) -> p k h d", h=HEADS)
                sc = spool.tile([128, kt, HEADS], dt.float32, tag="sc")
                for h in range(HEADS):
                    ph = int(p1[h])
                    scr = scrpool.tile([128, kt, HID], dt.bfloat16, tag="scr")
                    if ph > 0:
                        nc.scalar.activation(scr[:, :, 0:ph], et4[:, :, h, 0:ph],
                                             AF.Prelu, alpha=0.2)
                    if ph < HID:
                        nc.scalar.activation(scr[:, :, ph:HID], et4[:, :, h, ph:HID],
                                             AF.Prelu, scale=0.2, alpha=5.0)
                    nc.vector.tensor_reduce(sc[:, :, h], scr[:],
                                            axis=AX.X, op=ALU.add)

                # w = exp(score) * mask ; denom; 1/denom
                nc.scalar.activation(sc[:], sc[:], AF.Exp)
                nc.vector.tensor_tensor(
                    out=sc[:], in0=sc[:],
                    in1=mk[:].rearrange("p (k o) -> p k o", o=1).to_broadcast([128, kt, HEADS]),
                    op=ALU.mult)
                dn = spool.tile([128, HEADS], dt.float32, tag="dn")
                nc.vector.tensor_reduce(dn[:], sc[:].rearrange("p k h -> p h k"),
                                        axis=AX.X, op=ALU.add)
                nc.vector.tensor_scalar_add(dn[:], dn[:], 1e-12)
                rc = spool.tile([128, HEADS], dt.float32, tag="rc")
                nc.vector.reciprocal(rc[:], dn[:])

                # v = xl'[src] * w ; aggregate over slots
                nc.vector.tensor_tensor(
                    out=et[:], in0=et[:],
                    in1=sc[:].rearrange("p k (h o) -> p k h o", o=1).to_broadcast([128, kt, HEADS, HID]),
                    op=ALU.mult)
                ag = wpool.tile([128, F1], dt.float32, tag="ag")
                nc.vector.tensor_reduce(
                    ag[:], et[:].rearrange("p k (h d) -> p h d k", h=HEADS),
                    axis=AX.X, op=ALU.add)

                # unscale: * (1/denom) per head, * inv_att, + b1, then ELU
                nc.vector.tensor_tensor(
                    out=ag[:], in0=ag[:].rearrange("p (h d) -> p h d", h=HEADS),
                    in1=rc[:].rearrange("p (h o) -> p h o", o=1).to_broadcast([128, HEADS, HID]),
                    op=ALU.mult)
                nc.gpsimd.tensor_sub(ag[:], ag[:], xr[:])
                nc.gpsimd.tensor_mul(ag[:], ag[:], cst[:, oIA1:oIA1 + F1])
                nc.gpsimd.tensor_add(ag[:], ag[:], cst[:, oB1:oB1 + F1])
                # elu: max(x,0)-1 + exp(min(x,0))
                t1 = wpool.tile([128, F1], dt.float32, tag="elu1")
                nc.vector.tensor_scalar(t1[:], ag[:], 0.0, None, ALU.min)
                nc.scalar.activation(t1[:], t1[:], AF.Exp)
                nc.vector.tensor_scalar(ag[:], ag[:], 0.0, -1.0, ALU.max, ALU.add)
                nc.vector.tensor_add(ag[:], ag[:], t1[:])

                # transpose h -> hT tiles (SBUF) and L2 transforms right here
                tb = wpool.tile([128, S1, 128], dt.float32, tag="tb")
                for s in range(S1):
                    tp = ps2pool.tile([128, 128], dt.float32, space="PSUM", tag="tp")
                    nc.tensor.transpose(out=tp[:], in_=ag[:, s * 128:(s + 1) * 128],
                                        identity=ident[:])
                    nc.scalar.copy(out=tb[:, s, :], in_=tp[:])
                ps_l = ps2pool.tile([128, F2], dt.float32, space="PSUM", tag="l2l")
                ps_r = ps2pool.tile([128, F2], dt.float32, space="PSUM", tag="l2r")
                for s in range(S1):
                    nc.tensor.matmul(ps_l[:], lhsT=tb[:, s, :], rhs=w2l_s[:, s * F2:(s + 1) * F2],
                                     start=(s == 0), stop=(s == S1 - 1))
                    nc.tensor.matmul(ps_r[:], lhsT=tb[:, s, :], rhs=w2r_s[:, s * F2:(s + 1) * F2],
                                     start=(s == 0), stop=(s == S1 - 1))
                xl2 = wpool.tile([128, F2], dt.float32, tag="xl2")
                nc.scalar.copy(out=xl2[:], in_=ps_l[:])
                nc.vector.tensor_copy(out=xr2[:, t, :], in_=ps_r[:])
                nc.scalar.dma_start(out=cc_in.ap()[sl, :], in_=xl2[:])

                # chunk boundary: kick off the partial AllGather
                for g in range(NCHUNK):
                    if t == cb[g + 1] - 1:
                        r0, r1 = cb[g] * 128, cb[g + 1] * 128
                        if os.environ.get("KNOCC") == "1":
                            cbt = wpool.tile([128, F2], dt.float32, tag="ccstub")
                            for tt in range(cb[g], cb[g + 1]):
                                ssl = slice(tt * 128, (tt + 1) * 128)
                                nc.sync.dma_start(out=cbt[:], in_=cc_in.ap()[ssl, :])
                                nc.sync.dma_start(
                                    out=cc_sc[g].ap()[(tt - cb[g]) * 128:(tt - cb[g] + 1) * 128, :],
                                    in_=cbt[:])
                        else:
                            nc.gpsimd.collective_compute(
                                "AllGather", mybir.AluOpType.bypass,
                                replica_groups=[list(range(NCORES))],
                                ins=[cc_in.ap()[r0:r1, :].opt()],
                                outs=[cc_sc[g].ap().opt()])
                        rows = (cb[g + 1] - cb[g]) * 128
                        for c in range(NCORES):
                            nc.sync.dma_start(
                                out=cc_out.ap()[c * PC + r0:c * PC + r0 + rows, :],
                                in_=cc_sc[g].ap()[c * rows:(c + 1) * rows, :])

            tc.strict_bb_all_engine_barrier()

            # ---- phase 5: L2 edge processing + final linear ----
            lowT2 = cc_out.ap()[0:SPLIT, :]
            highT2 = cc_out.ap()[SPLIT:NG, :]
            pp2 = int(p2[0])
            for t in range(T if PH >= 5 else 0):
                kl, kh, kt = int(kLp[t]), int(kHp[t]), int(Kt[t])
                sl = slice(t * 128, (t + 1) * 128)
                et = e2pool.tile([128, kt, F2], dt.float32, tag="edge2")
                ax = spool.tile([128, 8 * kl + 8 * kh + kt], dt.int16, tag="ax")
                nc.sync.dma_start(out=ax[:], in_=auxd.ap()[:, int(offA[t]):int(offA[t + 1])])
                il = ax[:, 0:8 * kl]
                ih = ax[:, 8 * kl:8 * (kl + kh)]
                mk = ax[:, 8 * (kl + kh):].bitcast(dt.bfloat16)
                for c0 in range(0, kl, GCH2):
                    w = min(GCH2, kl - c0)
                    nc.gpsimd.dma_gather(et[:, c0:c0 + w, :], lowT2,
                                         il[:, c0 * 8:(c0 + w) * 8],
                                         w * 128, nreg(w * 128), F2)
                for c0 in range(0, kh, GCH2):
                    w = min(GCH2, kh - c0)
                    nc.gpsimd.dma_gather(et[:, kl + c0:kl + c0 + w, :], highT2,
                                         ih[:, c0 * 8:(c0 + w) * 8],
                                         w * 128, nreg(w * 128), F2)

                nc.vector.tensor_tensor(
                    out=et[:], in0=et[:],
                    in1=xr2[:, t, :].rearrange("p (o f) -> p o f", o=1).to_broadcast([128, kt, F2]),
                    op=ALU.add)

                scr = scrpool.tile([128, kt, F2], dt.float32, tag="scr2")
                if pp2 > 0:
                    nc.scalar.activation(scr[:, :, 0:pp2], et[:, :, 0:pp2],
                                         AF.Prelu, alpha=0.2)
                if pp2 < F2:
                    nc.scalar.activation(scr[:, :, pp2:F2], et[:, :, pp2:F2],
                                         AF.Prelu, scale=0.2, alpha=5.0)
                sc = spool.tile([128, kt], dt.float32, tag="sc2")
                nc.vector.tensor_reduce(sc[:], scr[:], axis=AX.X, op=ALU.add)
                nc.scalar.activation(sc[:], sc[:], AF.Exp)
                nc.vector.tensor_mul(sc[:], sc[:], mk[:])
                dn = spool.tile([128, 1], dt.float32, tag="dn2")
                nc.vector.tensor_reduce(dn[:], sc[:], axis=AX.X, op=ALU.add)
                nc.vector.tensor_scalar_add(dn[:], dn[:], 1e-12)
                rc = spool.tile([128, 1], dt.float32, tag="rc2")
                nc.vector.reciprocal(rc[:], dn[:])

                nc.vector.tensor_tensor(
                    out=et[:], in0=et[:],
                    in1=sc[:].rearrange("p (k o) -> p k o", o=1).to_broadcast([128, kt, F2]),
                    op=ALU.mult)
                ag = wpool.tile([128, F2], dt.float32, tag="ag2")
                nc.vector.tensor_reduce(
                    ag[:], et[:].rearrange("p k f -> p f k"), axis=AX.X, op=ALU.add)

                nc.vector.tensor_scalar_mul(ag[:], ag[:], rc[:, 0:1])
                nc.gpsimd.tensor_sub(ag[:], ag[:], xr2[:, t, :])
                nc.gpsimd.tensor_mul(ag[:], ag[:], cst[:, oIA2:oIA2 + F2])
                nc.gpsimd.tensor_add(ag[:], ag[:], cst[:, oB2:oB2 + F2])
                t1 = wpool.tile([128, F2], dt.float32, tag="elu2")
                nc.vector.tensor_scalar(t1[:], ag[:], 0.0, None, ALU.min)
                nc.scalar.activation(t1[:], t1[:], AF.Exp)
                nc.vector.tensor_scalar(ag[:], ag[:], 0.0, -1.0, ALU.max, ALU.add)
                nc.vector.tensor_add(ag[:], ag[:], t1[:])

                # final linear
                ot = spool.tile([128, OUT], dt.float32, tag="ot")
                tmp = wpool.tile([128, F2], dt.float32, tag="fl")
                for c in range(OUT):
                    nc.vector.tensor_mul(tmp[:], ag[:], cst[:, oWL + c * F2:oWL + (c + 1) * F2])
                    nc.vector.tensor_reduce(ot[:, c:c + 1], tmp[:], axis=AX.X, op=ALU.add)
                nc.vector.tensor_add(ot[:], ot[:], cst[:, oBL:oBL + OUT])
                nc.scalar.dma_start(out=out_d.ap()[sl, :], in_=ot[:])

    nc.compile()
    return nc


# --------------------------------------------------------------------------
# kernel() entry point
# --------------------------------------------------------------------------

_CACHE = {}


def kernel(x, edge_index, W1l, W1r, att1, b1, W2l, W2r, att2, b2, Wlin, blin):
    from concourse import bass_utils

    x = np.asarray(x, np.float32)
    N, IN_DIM = x.shape
    HEADS, HID = np.asarray(att1).shape
    F1 = HEADS * HID
    OUT_DIM = np.asarray(Wlin).shape[1]
    E = edge_index.shape[1]
    src = np.asarray(edge_index[0], np.int64)
    dst = np.asarray(edge_index[1], np.int64)

    PC = -(-(-(-N // -NCORES)) // -128) * 128  # ceil(ceil(N/8)/128)*128
    PC = ((N + NCORES - 1) // NCORES + 127) // 128 * 128
    T = PC // 128
    NG = NCORES * PC
    SPLIT = NLOW_CORES * PC
    assert SPLIT < 32768 and NG - SPLIT < 32768

    # ---- node -> core assignment + permutation ----
    deg = np.bincount(dst, minlength=N)
    assert deg.min() >= 1, "degree-0 nodes present; kernel assumes none"
    order0 = np.argsort(deg, kind="stable")
    core_of = np.empty(N, np.int64)
    core_of[order0] = np.arange(N) % NCORES
    # low/high source
    is_low_src = core_of[src] < NLOW_CORES
    kL0 = np.zeros(N, np.int64)
    kH0 = np.zeros(N, np.int64)
    np.add.at(kL0, dst[is_low_src], 1)
    np.add.at(kH0, dst[~is_low_src], 1)
    # per-core sort by (kL, kH)
    perm_lists = []
    pos_of = np.empty(N, np.int64)
    for c in range(NCORES):
        nodes = np.flatnonzero(core_of == c)
        o = np.lexsort((kH0[nodes], kL0[nodes]))
        nodes = nodes[o]
        perm_lists.append(nodes)
        pos_of[nodes] = c * PC + np.arange(len(nodes))

    src_pos = pos_of[src]
    dst_pos = pos_of[dst]

    meta, aux = _prep_graph(src_pos, dst_pos, N, PC, T)

    # ---- weights (host-side param prep: att folding + sign-grouping) ----
    att1_f = np.asarray(att1, np.float64).reshape(-1)
    att2_f = np.asarray(att2, np.float64).reshape(-1)
    assert np.abs(att1_f).min() > 1e-12 and np.abs(att2_f).min() > 1e-12
    pi1, p1 = _sign_perm(att1_f, HEADS, HID)
    pi2, p2 = _sign_perm(att2_f, 1, HID)

    W1l_f = (np.asarray(W1l, np.float64) * att1_f[None, :])[:, pi1].astype(np.float32)
    W1r_f = (np.asarray(W1r, np.float64) * att1_f[None, :])[:, pi1].astype(np.float32)
    inv1 = (1.0 / att1_f)[pi1].astype(np.float32)
    b1_p = np.asarray(b1, np.float32)[pi1]
    W2l_f = ((np.asarray(W2l, np.float64)[pi1, :]) * att2_f[None, :])[:, pi2].astype(np.float32)
    W2r_f = ((np.asarray(W2r, np.float64)[pi1, :]) * att2_f[None, :])[:, pi2].astype(np.float32)
    inv2 = (1.0 / att2_f)[pi2].astype(np.float32)
    b2_p = np.asarray(b2, np.float32)[pi2]
    Wlin_p = np.asarray(Wlin, np.float32)[pi2, :]
    blin_p = np.asarray(blin, np.float32)

    S1 = F1 // 128
    w2l_dev = W2l_f.reshape(S1, 128, HID).transpose(1, 0, 2).reshape(128, S1 * HID)
    w2r_dev = W2r_f.reshape(S1, 128, HID).transpose(1, 0, 2).reshape(128, S1 * HID)

    CW = 2 * F1 + 2 * HID + OUT_DIM * HID + OUT_DIM
    consts = np.zeros((128, CW), np.float32)
    consts[:, 0:F1] = inv1[None, :]
    consts[:, F1:2 * F1] = b1_p[None, :]
    consts[:, 2 * F1:2 * F1 + HID] = inv2[None, :]
    consts[:, 2 * F1 + HID:2 * F1 + 2 * HID] = b2_p[None, :]
    for c in range(OUT_DIM):
        consts[:, 2 * F1 + 2 * HID + c * HID:2 * F1 + 2 * HID + (c + 1) * HID] = Wlin_p[:, c][None, :]
    consts[:, 2 * F1 + 2 * HID + OUT_DIM * HID:] = blin_p[None, :]

    # permuted x, padded + transposed
    x_perm = np.zeros((NG, IN_DIM), np.float32)
    for c in range(NCORES):
        nodes = perm_lists[c]
        x_perm[c * PC:c * PC + len(nodes)] = x[nodes]
    xT_full = np.ascontiguousarray(x_perm.T)

    cfg = dict(NG=NG, PC=PC, T=T, SPLIT=SPLIT, IN_DIM=IN_DIM, HID=HID,
               HEADS=HEADS, OUT_DIM=OUT_DIM, CW=CW,
               kLp=meta["kLp"], kHp=meta["kHp"], Kt=meta["Kt"],
               offL=meta["offL"], offH=meta["offH"], offM=meta["offM"],
               offA=meta["offA"], p1=p1, p2=p2)

    key = (N, E, IN_DIM, HID, HEADS, OUT_DIM,
           tuple(cfg["kLp"]), tuple(cfg["kHp"]), tuple(p1), tuple(p2))
    if key not in _CACHE:
        _CACHE[key] = build_program(cfg)
    nc = _CACHE[key]

    in_maps = []
    for c in range(NCORES):
        in_maps.append({
            "xT": xT_full,
            "xTo": np.ascontiguousarray(xT_full[:, c * PC:(c + 1) * PC]),
            "w1l": W1l_f, "w1r": W1r_f, "w2l": w2l_dev, "w2r": w2r_dev,
            "consts": consts,
            "auxd": aux[c],
        })

    res = bass_utils.run_bass_kernel_spmd(nc, in_maps, core_ids=list(range(NCORES)))
    kernel._last = dict(res=res.results, in_maps=in_maps, nc=nc,
                        perm_lists=perm_lists, pos_of=pos_of,
                        cfg=cfg, meta=meta, W1l_f=W1l_f, W1r_f=W1r_f,
                        pi1=pi1, pi2=pi2, inv1=inv1, x_perm=x_perm, aux=aux)

    out = np.empty((N, OUT_DIM), np.float32)
    for c in range(NCORES):
        nodes = perm_lists[c]
        out[nodes] = res.results[c]["out"][:len(nodes)]
    return out
